# revision 15
# baseline (speedup 1.0000x reference)
"""Hypernetwork causal attention (nn_Attention_87926570484382) on 8 TRN2 cores.

Strategy (two launches, batch-sharded attention, gen-sharded hypernet):
  host   : time-embedding MLP -> t [128]; bias assembly; q-scale folding.
  launch1: each core streams 1/8 of fW_attn_w / fW_proj_w through the PE
           (stationary = t replicated across 128 cols, f32r) producing its
           slice of W_attn / W_proj.
  host   : gather W slices, add biases, fold 1/sqrt(D) into the q columns.
  launch2: each core runs full attention for 2 of the 16 batches:
           qk^T = W_attn^T-tiles @ x^T   (heads on partitions)
           scores^T[k,q] per (b,h), exp (no max-sub; causal mask as
           multiply on diagonal tiles), attn@V via ones-extended V (softmax
           denominators land in psum rows 64..127), per-head normalize,
           proj with bias via K=1 matmul, out[s,e].

All matmuls run in float32r (TF32-like, 1 cycle/row); rel err ~2e-4.
"""

import os
import sys

import numpy as np

# ---------------------------------------------------------------------------
# Environment shims (must precede concourse imports in fresh environments)
# ---------------------------------------------------------------------------


def _ensure_axon_hooks():
    """Provide antenv.axon_hooks if the installed antenv lacks it (needed
    only when tracing; harmless otherwise)."""
    try:
        import antenv.axon_hooks  # noqa: F401
        return
    except ImportError:
        pass
    try:
        import antenv
    except ImportError:
        return
    import contextlib
    import ctypes
    import types

    mod = types.ModuleType("antenv.axon_hooks")
    mod._HOOK = None
    mod._TRIED = False

    def set_axon_ntff_profile_hook(hook):
        mod._HOOK = hook

    def _build(so_path):
        lib = ctypes.CDLL(so_path)
        if not hasattr(lib, "axon_start_nrt_profile"):
            return None
        lib.axon_start_nrt_profile.argtypes = [
            ctypes.POINTER(ctypes.c_int64),
            ctypes.c_size_t,
        ]
        lib.axon_start_nrt_profile.restype = ctypes.c_int64
        lib.axon_stop_nrt_profile.argtypes = [ctypes.c_char_p]
        lib.axon_stop_nrt_profile.restype = ctypes.c_int64

        @contextlib.contextmanager
        def _hook(output_dir, device_ids):
            import jax

            jax.devices()
            if device_ids:
                ids = (ctypes.c_int64 * len(device_ids))(*device_ids)
                rc = lib.axon_start_nrt_profile(ids, len(device_ids))
            else:
                rc = lib.axon_start_nrt_profile(None, 0)
            if rc != 0:
                raise RuntimeError(f"axon_start_nrt_profile rc={rc}")
            try:
                yield
            finally:
                n = lib.axon_stop_nrt_profile(str(output_dir).encode())
                print(f"profile: {n} file(s) -> {output_dir}", file=sys.stderr)

        return _hook

    def get_axon_ntff_profile_hook():
        if mod._HOOK is None and not mod._TRIED:
            mod._TRIED = True
            p = "/opt/axon/libaxon_pjrt.so"
            if os.path.exists(p):
                try:
                    mod._HOOK = _build(p)
                except OSError:
                    mod._HOOK = None
        return mod._HOOK

    mod.set_axon_ntff_profile_hook = set_axon_ntff_profile_hook
    mod.get_axon_ntff_profile_hook = get_axon_ntff_profile_hook
    sys.modules["antenv.axon_hooks"] = mod
    antenv.axon_hooks = mod


_ensure_axon_hooks()

import concourse.bass as bass  # noqa: E402
import concourse.mybir as mybir  # noqa: E402
from concourse import tile as _tile  # noqa: E402
from concourse.tile import TileContext, add_dep_helper  # noqa: E402
from concourse.vector_clock import ScopedClock  # noqa: E402
from concourse.bass_utils import run_bass_kernel_spmd  # noqa: E402

F32 = mybir.dt.float32
F32R = mybir.dt.float32r
F16 = mybir.dt.float16
FP8 = mybir.dt.float8e4

# problem constants (hardcoded per harness contract)
SIN_DIM, TEMBED = 64, 128
E, H, D = 512, 8, 64
B, S = 16, 1024
NCORES = 8
SKIP_NR = os.environ.get("SKIP_NR", "1") == "1"
BPC = B // NCORES          # batches per core
S2 = BPC * S               # 2048 rows per core
J3 = 3 * H * D             # 1536
EJ_A = E * J3 // NCORES    # 98304  fW_attn slab cols per core
EJ_P = E * E // NCORES     # 32768  fW_proj slab cols per core
GTOT = (E * J3 + E * E) // NCORES   # 131072 slab cols per core (concat split)
GT_TILES = GTOT // 128              # 1024 stationary tiles per core
NT = 2                              # t columns per matmul (hi, lo fp8)
GROUP_TILES = 512 // NT             # tiles per psum bank

# ---------------------------------------------------------------------------
# Tile framework workarounds: this walrus accepts at most ONE semaphore wait
# and one update per instruction.
# ---------------------------------------------------------------------------

_NOP_CTR = [0]


def _patched_drain_and_barrier(self, tick_clock, wait_clock):
    carrier = self.nc.sync.nop(nofuse=True)
    wait_clock.add_sem_waits(
        carrier.ins, ScopedClock({None: tick_clock.global_clock})
    )
    si = carrier.ins.sync_info
    waits = list(si.on_wait) if si and si.on_wait else []
    if len(waits) > 1:
        carrier.ins.sync_info = mybir.SyncInfo(
            on_wait=waits[:1],
            on_update=list(si.on_update) if si and si.on_update else [],
        )
        for w in waits[1:]:
            extra = self.nc.sync.nop(nofuse=True)
            extra.ins.sync_info = mybir.SyncInfo(on_wait=[w], on_update=[])
    self.nc.sync.drain()
    self.nc.all_engine_barrier()
    assert self.sems is not None
    popped = self.nc._tile_sem_poison_stack.pop()
    assert popped is self._sem_poison
    self.nc.clear_and_free_semaphores(list(self.sems.allocated().values()))
    self.nc.all_engine_barrier()


_tile.TileContext._drain_and_barrier = _patched_drain_and_barrier


def _split_multi_waits(nc):
    for f in nc.m.functions:
        for blk in f.blocks:
            out = []
            changed = False
            for inst in blk.instructions:
                si = inst.sync_info
                waits = list(si.on_wait) if si and si.on_wait else []
                updates = list(si.on_update) if si and si.on_update else []
                is_dma = "DMA" in type(inst).__name__
                if len(waits) > 1:
                    changed = True
                    for w in waits[:-1]:
                        _NOP_CTR[0] += 1
                        nop = mybir.InstNoOp(
                            name=f"wsplit_{_NOP_CTR[0]}", ins=[], outs=[]
                        )
                        nop.engine = inst.engine
                        nop.sync_info = mybir.SyncInfo(on_wait=[w], on_update=[])
                        out.append(nop)
                    waits = [waits[-1]]
                    inst.sync_info = mybir.SyncInfo(
                        on_wait=waits, on_update=updates
                    )
                out.append(inst)
                if len(updates) > 1:
                    if is_dma:
                        raise AssertionError(
                            f"DMA {inst.name} has {len(updates)} updates"
                        )
                    changed = True
                    inst.sync_info = mybir.SyncInfo(
                        on_wait=waits, on_update=[updates[0]]
                    )
                    for u in updates[1:]:
                        _NOP_CTR[0] += 1
                        nop = mybir.InstNoOp(
                            name=f"usplit_{_NOP_CTR[0]}", ins=[], outs=[]
                        )
                        nop.engine = inst.engine
                        nop.sync_info = mybir.SyncInfo(on_wait=[], on_update=[u])
                        out.append(nop)
            if changed:
                blk.instructions = out
    return nc


# ---------------------------------------------------------------------------
# Launch 1: hypernetwork weight generation
# ---------------------------------------------------------------------------


def build_gen():
    """Per core: W_slab[c] = sum_k t[k] * fW_slab[k, c] for a 1/8 slice of
    the concatenated (fW_attn_w | fW_proj_w) column space.

    Flipped matmul orientation: the fp8 fW tile [128k, 128j] is the
    STATIONARY operand (FWL-accelerated LDWEIGHTS carries the bandwidth)
    and t is the moving operand ([128, 2]: hi/lo fp8 split), so each
    matmul emits a compact [128, 2] psum column pair -- no replicated
    output, copies shrink from 64 MB to 1 MB per core.  Host combines
    hi + lo/16 and rescales."""
    nc = bass.Bass()
    fw = nc.dram_tensor("fw", [TEMBED, GTOT], FP8, kind="ExternalInput")
    tv = nc.dram_tensor("tv", [TEMBED, NT], FP8, kind="ExternalInput")
    g = nc.dram_tensor("g", [128, GT_TILES * NT], F32, kind="ExternalOutput")

    GCHUNK = 8192           # fW cols per DMA chunk (1 MB fp8)
    TPC = GCHUNK // 128     # 64 stationary tiles per chunk

    with TileContext(nc) as tc:
        with (
            tc.tile_pool(name="cst", bufs=1) as cst,
            tc.tile_pool(name="sb", bufs=5) as sb,
            tc.tile_pool(name="gs", bufs=2) as gsp,
            tc.tile_pool(name="ps", bufs=2, space="PSUM") as ps,
        ):
            tt = cst.tile([TEMBED, NT], FP8)
            nc.sync.dma_start(out=tt, in_=tv[:, :])

            pg = None
            for chunk in range(GTOT // GCHUNK):
                ft = sb.tile([TEMBED, GCHUNK], FP8, tag="fw", name=f"fw{chunk}")
                # alternate the two HWDGE rings so fixed DMA latencies overlap
                eng = nc.sync if chunk % 2 == 0 else nc.scalar
                eng.dma_start(
                    out=ft, in_=fw[:, chunk * GCHUNK : (chunk + 1) * GCHUNK]
                )
                for i in range(TPC):
                    c = chunk * TPC + i
                    grp, off = divmod(c, GROUP_TILES)
                    if off == 0:
                        pg = ps.tile([128, 512], F32, tag="pg", name=f"pg{grp}")
                    nc.tensor.matmul(
                        pg[:, NT * off : NT * (off + 1)],
                        ft[:, 128 * i : 128 * (i + 1)],
                        tt,
                        start=True,
                        stop=True,
                    )
                    if off == GROUP_TILES - 1:
                        gt = gsp.tile([128, 512], F32, tag="gs", name=f"gs{grp}")
                        nc.vector.tensor_copy(gt, pg)
                        # outputs ride SWDGE so they never head-of-line
                        # block the input chunk stream
                        nc.gpsimd.dma_start(
                            out=g[:, 512 * grp : 512 * (grp + 1)], in_=gt
                        )
    _split_multi_waits(nc)
    return nc


# ---------------------------------------------------------------------------
# Launch 2: attention for 2 batches per core
# ---------------------------------------------------------------------------


def _act_recip_seed(nc, out, in_):
    eng = nc.scalar
    imm = lambda v: mybir.ImmediateValue(dtype=mybir.dt.float32, value=v)
    return eng.add_instruction(
        mybir.InstActivation(
            name=eng.bass.get_next_instruction_name(),
            func=mybir.ActivationFunctionType.Reciprocal,
            ins=[eng.lower_ap(in_), imm(0.0), imm(1.0), imm(0.0)],
            outs=[eng.lower_ap(out)],
        )
    )


def build_attn():
    nc = bass.Bass()
    xt = nc.dram_tensor("xt", [E, S2], F16, kind="ExternalInput")
    wa = nc.dram_tensor("wa", [E, J3], F16, kind="ExternalInput")
    wp = nc.dram_tensor("wp", [E, E], F16, kind="ExternalInput")
    bqk = nc.dram_tensor("bqk", [128, 8], F32, kind="ExternalInput")
    brow = nc.dram_tensor("brow", [1, E], F16, kind="ExternalInput")
    mask = nc.dram_tensor("mask", [128, 128], F16, kind="ExternalInput")
    ones = nc.dram_tensor("ones", [128, 128], F16, kind="ExternalInput")
    out = nc.dram_tensor("out", [S2, E], F32, kind="ExternalOutput")

    NQT = S // 128           # 8 q/k tiles per batch
    NET = E // 128           # 4 e tiles

    with TileContext(nc) as tc:
        with (
            tc.tile_pool(name="cst", bufs=1) as cst,
            tc.tile_pool(name="qk", bufs=1) as qkp,
            tc.tile_pool(name="vx", bufs=1) as vxp,
            tc.tile_pool(name="ot", bufs=1) as otp,
            tc.tile_pool(name="ou", bufs=1) as oup,
            tc.tile_pool(name="wk", bufs=3) as wk,
            tc.tile_pool(name="ex", bufs=4) as exp_pool,
            tc.tile_pool(name="ps", bufs=3, space="PSUM") as ps,
            tc.tile_pool(name="po", bufs=5, space="PSUM") as pop,
        ):
            # resident inputs
            xts = []
            was = []
            wps = []
            for et in range(NET):
                t_ = cst.tile([128, S2], F16, tag=f"xt{et}")
                nc.sync.dma_start(out=t_, in_=xt[128 * et : 128 * (et + 1), :])
                xts.append(t_)
                t_ = cst.tile([128, J3], F16, tag=f"wa{et}")
                nc.sync.dma_start(out=t_, in_=wa[128 * et : 128 * (et + 1), :])
                was.append(t_)
                t_ = cst.tile([128, E], F16, tag=f"wp{et}")
                nc.sync.dma_start(out=t_, in_=wp[128 * et : 128 * (et + 1), :])
                wps.append(t_)
            bqk_t = cst.tile([128, 8], F32)
            nc.sync.dma_start(out=bqk_t, in_=bqk[:, :])
            brow_t = cst.tile([1, E], F16)
            nc.sync.dma_start(out=brow_t, in_=brow[:, :])
            mask_t = cst.tile([128, 128], F16)
            nc.sync.dma_start(out=mask_t, in_=mask[:, :])
            ones1 = cst.tile([1, 128], F16)
            nc.sync.dma_start(out=ones1, in_=ones[0:1, :])
            # persistent v_ext tiles: [128 k-rows, 8h * (64 v | 64 ones)];
            # ones columns are constant, v columns rewritten per batch
            vxs = []
            for st in range(S // 128):
                v_ = vxp.tile([128, 8 * 128], F16, tag=f"vx{st}", name=f"vx{st}")
                for h in range(H):
                    nc.sync.dma_start(
                        out=v_[:, 128 * h + 64 : 128 * (h + 1)],
                        in_=ones[:, 0:64],
                    )
                vxs.append(v_)

            state = {}

            def qkv_phase(b):
                s0 = b * S
                qkts = []
                for m in range(8):
                    qkts.append(qkp.tile([128, S], F16, tag=f"qkT{m}", name=f"qkT{m}_{b}"))
                for m in range(8):
                    for sc in range(S // 512):
                        pq = ps.tile([128, 512], F32, tag="ps", name=f"pq_{b}_{m}_{sc}")
                        for et in range(NET):
                            nc.tensor.matmul(
                                pq,
                                was[et][:, 128 * m : 128 * (m + 1)],
                                xts[et][:, s0 + 512 * sc : s0 + 512 * (sc + 1)],
                                start=(et == 0),
                                stop=(et == NET - 1),
                            )
                        nc.vector.tensor_scalar_add(
                            qkts[m][:, 512 * sc : 512 * (sc + 1)],
                            pq,
                            bqk_t[:, m : m + 1],
                        )
                for st in range(NQT):
                    pv = ps.tile([128, 512], F32, tag="ps", name=f"pv_{b}_{st}")
                    for et in range(NET):
                        nc.tensor.matmul(
                            pv,
                            xts[et][:, s0 + 128 * st : s0 + 128 * (st + 1)],
                            was[et][:, 1024:1536],
                            start=(et == 0),
                            stop=(et == NET - 1),
                        )
                    for h in range(H):
                        if h % 2 == 0:
                            nc.vector.tensor_copy(
                                vxs[st][:, 128 * h : 128 * h + 64],
                                pv[:, 64 * h : 64 * (h + 1)],
                            )
                        else:
                            nc.scalar.copy(
                                vxs[st][:, 128 * h : 128 * h + 64],
                                pv[:, 64 * h : 64 * (h + 1)],
                            )
                state[b] = {"qkts": qkts}

            def attn_phase(b):
                st_ = state[b]
                qkts = st_["qkts"]
                ots = [
                    otp.tile([128, S], F16, tag=f"oT{ht}", name=f"oT{ht}_{b}")
                    for ht in range(NET)
                ]
                osm = [
                    oup.tile([128, S], F32, tag=f"ou{h}", name=f"ou{h}_{b}")
                    for h in range(H)
                ]
                st_.update(ots=ots, osm=osm, exps=[])
                # sandwich: this batch's exps must follow the previous
                # batch's reciprocal seeds (ACT table thrash avoidance)
                prev = state.get(b - 1)
                if prev and prev.get("seeds"):
                    st_["first_exp_dep"] = prev["seeds"][-1]
                for h in range(H):
                    kt_tile = qkts[4 + h // 2]
                    qt_tile = qkts[h // 2]
                    prow = 64 * (h % 2)
                    po0 = pop.tile([128, 512], F32, tag="po", name=f"po0_{b}_{h}")
                    po1 = pop.tile([128, 512], F32, tag="po", name=f"po1_{b}_{h}")
                    pos = (po0, po1)

                    def scores_exp(j):
                        expt = exp_pool.tile(
                            [128, S], F16, tag="expt", name=f"expt_{b}_{h}_{j}"
                        )
                        for qc in range(2):
                            c0 = max(512 * qc, 128 * j)
                            c1 = 512 * (qc + 1)
                            if c0 >= c1:
                                continue
                            pst = ps.tile(
                                [128, 512], F32, tag="ps", name=f"pst_{b}_{h}_{j}_{qc}"
                            )
                            nc.tensor.matmul(
                                pst[:, 0 : c1 - c0],
                                kt_tile[prow : prow + 64, 128 * j : 128 * (j + 1)],
                                qt_tile[prow : prow + 64, c0:c1],
                                start=True,
                                stop=True,
                            )
                            ei = nc.scalar.activation(
                                expt[:, c0:c1],
                                pst[:, 0 : c1 - c0],
                                func=mybir.ActivationFunctionType.Exp,
                            )
                            st_["exps"].append(ei.ins)
                        nc.vector.tensor_mul(
                            expt[:, 128 * j : 128 * (j + 1)],
                            expt[:, 128 * j : 128 * (j + 1)],
                            mask_t,
                        )
                        return expt

                    expts = {0: scores_exp(0), 1: scores_exp(1)}
                    for j in range(NQT):
                        if j + 2 < NQT:
                            expts[j + 2] = scores_exp(j + 2)
                        expt = expts.pop(j)
                        for qc in range(2):
                            c0 = max(512 * qc, 128 * j)
                            c1 = 512 * (qc + 1)
                            if c0 >= c1:
                                continue
                            nc.tensor.matmul(
                                pos[qc][:, c0 - 512 * qc : c1 - 512 * qc],
                                vxs[j][:, 128 * h : 128 * (h + 1)],
                                expt[:, c0:c1],
                                start=(j == 0),
                                stop=(j == NQT - 1 if qc == 1 else j == 3),
                            )
                    for qc in range(2):
                        nc.vector.tensor_copy(
                            osm[h][:, 512 * qc : 512 * (qc + 1)],
                            pos[qc][:, :],
                        )

            def norm_phase(b):
                st_ = state[b]
                ots, osm = st_["ots"], st_["osm"]
                y0s = {}
                st_["seeds"] = []
                for h in range(H):
                    y0s[h] = wk.tile([64, S], F32, tag="y0", name=f"y0_{b}_{h}")
                    si = _act_recip_seed(nc, y0s[h], osm[h][64:128, :])
                    st_["seeds"].append(si.ins)
                for h in range(H):
                    prow = 64 * (h % 2)
                    if SKIP_NR:
                        rec = y0s[h]
                    else:
                        tvar = wk.tile([64, S], F32, tag="tvar", name=f"tv_{b}_{h}")
                        nc.vector.tensor_mul(tvar, osm[h][64:128, :], y0s[h])
                        nc.vector.tensor_scalar(
                            out=tvar, in0=tvar, scalar1=-1.0, scalar2=2.0,
                            op0=mybir.AluOpType.mult, op1=mybir.AluOpType.add,
                        )
                        rec = wk.tile([64, S], F32, tag="rec", name=f"rec_{b}_{h}")
                        nc.vector.tensor_mul(rec, tvar, y0s[h])
                    nc.vector.tensor_mul(
                        ots[h // 2][prow : prow + 64, :], osm[h][0:64, :], rec
                    )

            def proj_phase(b, st_lo=0, st_hi=NQT):
                s0 = b * S
                ots = state[b]["ots"]
                for st in range(st_lo, st_hi):
                    pp = ps.tile([128, 512], F32, tag="ps", name=f"pp_{b}_{st}")
                    for ht in range(NET):
                        nc.tensor.matmul(
                            pp,
                            ots[ht][:, 128 * st : 128 * (st + 1)],
                            wps[ht],
                            start=(ht == 0),
                            stop=False,
                        )
                    nc.tensor.matmul(
                        pp, ones1, brow_t, start=False, stop=True
                    )
                    ob = wk.tile([128, E], F32, tag="ob", name=f"ob_{b}_{st}")
                    nc.vector.tensor_copy(ob, pp)
                    nc.sync.dma_start(
                        out=out[s0 + 128 * st : s0 + 128 * (st + 1), :], in_=ob
                    )

            # batch-level software pipeline: norm(b)/proj(b) overlap the
            # next batch's dense PE phases
            qkv_phase(0)
            attn_phase(0)
            qkv_phase(1)
            norm_phase(0)
            attn_phase(1)
            proj_phase(0, 0, 4)
            norm_phase(1)
            proj_phase(0, 4, NQT)
            proj_phase(1)
    _split_multi_waits(nc)
    return nc


# ---------------------------------------------------------------------------
# Launch 2 (v2): attention, restructured for HAM warmth + merged exp calls
# ---------------------------------------------------------------------------


def build_attn2():
    """Per core: full causal attention for 2 batches.

    vs v1: (1) head-pair score matmuls issued adjacently so the K=64 row
    tiles (partitions 0-63 / 64-127) run concurrently on the PE;
    (2) scores for a (pair, j) land in one 4-bank psum tile and are
    exponentiated by ONE ACT call via a [128, 2, N] access pattern
    (6 calls/pair instead of 28); (3) softmax normalize via DVE
    reciprocal_approx_fast straight out of psum (no Reciprocal table
    thrash, no replicated copies); (4) qkv/proj chunks of the
    neighbouring batch are interleaved into the pair loop to fill the
    PE gaps left by the exp dependency chain."""
    nc = bass.Bass()
    xt = nc.dram_tensor("xt", [E, S2], F16, kind="ExternalInput")
    wa = nc.dram_tensor("wa", [E, J3], F16, kind="ExternalInput")
    wp = nc.dram_tensor("wp", [E, E], F16, kind="ExternalInput")
    bqk = nc.dram_tensor("bqk", [128, 8], F32, kind="ExternalInput")
    brow = nc.dram_tensor("brow", [1, E], F16, kind="ExternalInput")
    mask = nc.dram_tensor("mask", [128, 128], F16, kind="ExternalInput")
    ones = nc.dram_tensor("ones", [128, 128], F16, kind="ExternalInput")
    out = nc.dram_tensor("out", [S2, E], F32, kind="ExternalOutput")

    NQT = S // 128           # 8 k-tiles (and q-tiles) per batch
    NET = E // 128           # 4 e-tiles

    with TileContext(nc) as tc:
        with (
            tc.tile_pool(name="cst", bufs=1) as cst,
            tc.tile_pool(name="qk", bufs=1) as qkp,
            tc.tile_pool(name="vx", bufs=1) as vxp,
            tc.tile_pool(name="ot", bufs=1) as otp,
            tc.tile_pool(name="ex", bufs=2) as exp_pool,
            tc.tile_pool(name="wk", bufs=3) as wk,
            tc.tile_pool(name="sp", bufs=1, space="PSUM") as spp,
            tc.tile_pool(name="po", bufs=2, space="PSUM") as pop,
            tc.tile_pool(name="ps", bufs=2, space="PSUM") as ps,
        ):
            # ---- streamed input residency ----
            xts = [
                cst.tile([128, S2], F16, tag=f"xt{et}", name=f"xts{et}")
                for et in range(NET)
            ]
            was = [
                cst.tile([128, J3], F16, tag=f"wa{et}", name=f"was{et}")
                for et in range(NET)
            ]
            wps = [
                cst.tile([128, E], F16, tag=f"wp{et}", name=f"wps{et}")
                for et in range(NET)
            ]
            # load order tuned for earliest first matmul: q|k weight cols,
            # then batch-0 x, then v cols, then batch-1 x, then proj weights
            bqk_t = cst.tile([128, 8], F32)
            nc.sync.dma_start(out=bqk_t, in_=bqk[:, :])
            for et in range(NET):
                nc.sync.dma_start(
                    out=was[et][:, 0:1024], in_=wa[128 * et : 128 * (et + 1), 0:1024]
                )
            for et in range(NET):
                nc.sync.dma_start(
                    out=xts[et][:, 0:S], in_=xt[128 * et : 128 * (et + 1), 0:S]
                )
            for et in range(NET):
                nc.sync.dma_start(
                    out=was[et][:, 1024:J3],
                    in_=wa[128 * et : 128 * (et + 1), 1024:J3],
                )
            mask_t = cst.tile([128, 128], F16)
            nc.sync.dma_start(out=mask_t, in_=mask[:, :])
            ones1 = cst.tile([1, 128], F16)
            nc.sync.dma_start(out=ones1, in_=ones[0:1, :])
            brow_t = cst.tile([1, E], F16)
            nc.sync.dma_start(out=brow_t, in_=brow[:, :])
            for et in range(NET):
                nc.sync.dma_start(
                    out=xts[et][:, S:S2], in_=xt[128 * et : 128 * (et + 1), S:S2]
                )
            for et in range(NET):
                nc.sync.dma_start(
                    out=wps[et], in_=wp[128 * et : 128 * (et + 1), :]
                )
            # v_ext tiles: [128 k, 8h * (64 v | 64 ones)]; per batch so the
            # next batch's v generation never WAR-blocks on this batch's
            # attn@V reads.  ones cols are constant per tile.
            vxs = {}
            for b in range(BPC):
                vxs[b] = []
                for st in range(NQT):
                    v_ = vxp.tile(
                        [128, 8 * 128], F16, tag=f"vx{st}_{b}", name=f"vx{st}_{b}"
                    )
                    for h in range(H):
                        nc.sync.dma_start(
                            out=v_[:, 128 * h + 64 : 128 * (h + 1)],
                            in_=ones[:, 0:64],
                        )
                    vxs[b].append(v_)

            state = {}

            # ---- phase pieces, emitted in interleaved order below ----

            def qkv_qk_chunk(b, m, sc):
                """qk^T[128 cols of m-chunk, 512 s] for batch b."""
                s0 = b * S
                qkts = state[b]["qkts"]
                pq = ps.tile([128, 512], F32, tag="ps", name=f"pq_{b}_{m}_{sc}")
                for et in range(NET):
                    nc.tensor.matmul(
                        pq,
                        was[et][:, 128 * m : 128 * (m + 1)],
                        xts[et][:, s0 + 512 * sc : s0 + 512 * (sc + 1)],
                        start=(et == 0),
                        stop=(et == NET - 1),
                    )
                nc.vector.tensor_scalar_add(
                    qkts[m][:, 512 * sc : 512 * (sc + 1)],
                    pq,
                    bqk_t[:, m : m + 1],
                )

            def qkv_v_chunk(b, st):
                """v rows for s-tile st of batch b, scattered into vxs."""
                s0 = b * S
                pv = ps.tile([128, 512], F32, tag="ps", name=f"pv_{b}_{st}")
                for et in range(NET):
                    nc.tensor.matmul(
                        pv,
                        xts[et][:, s0 + 128 * st : s0 + 128 * (st + 1)],
                        was[et][:, 1024:1536],
                        start=(et == 0),
                        stop=(et == NET - 1),
                    )
                # one strided copy: [128, 8, 64] psum -> v cols of vxs[b][st]
                nc.vector.tensor_copy(
                    vxs[b][st][:, 0 : 8 * 128].rearrange(
                        "p (h t) -> p h t", h=8
                    )[:, :, 0:64],
                    pv.rearrange("p (h d) -> p h d", h=8),
                )

            def scores_exp(b, p, j):
                """Scores + exp for head pair p, k-tile j: two concurrent
                K=64 matmul chains into one 4-bank psum tile, one exp."""
                qkts = state[b]["qkts"]
                kt, qt = qkts[4 + p], qkts[p]
                c0 = 128 * j
                n = S - c0
                sp = spp.tile([128, 2048], F32, tag="spair", name=f"sp_{b}_{p}_{j}")
                for lo, hi in ((c0, 512), (max(512, c0), 1024)):
                    if lo >= hi:
                        continue
                    for hh in range(2):
                        nc.tensor.matmul(
                            sp[:, 1024 * hh + lo : 1024 * hh + hi],
                            kt[64 * hh : 64 * hh + 64, c0 : c0 + 128],
                            qt[64 * hh : 64 * hh + 64, lo:hi],
                            start=True,
                            stop=True,
                        )
                ex = exp_pool.tile(
                    [128, 2 * n], F16, tag=f"ex{j}", name=f"ex_{b}_{p}_{j}"
                )
                nc.scalar.activation(
                    ex.rearrange("p (h q) -> p h q", h=2),
                    sp.rearrange("p (h q) -> p h q", h=2)[:, :, c0:S],
                    func=mybir.ActivationFunctionType.Exp,
                )
                # causal mask on the diagonal 128-col block of both heads
                nc.vector.tensor_mul(
                    ex.rearrange("p (h q) -> p h q", h=2)[:, :, 0:128],
                    ex.rearrange("p (h q) -> p h q", h=2)[:, :, 0:128],
                    mask_t.unsqueeze(1).broadcast_to((128, 2, 128)),
                )
                state[b][f"ex_{p}_{j}"] = ex

            def attnv_norm(b, p):
                """attn @ V_ext for pair p (2 q-passes), then normalize into
                the ots tile rows of each head."""
                ots = state[b]["ots"]
                for qc in range(2):
                    pos = {}
                    for hh in range(2):
                        po = pop.tile(
                            [128, 512], F32, tag="po", name=f"po_{b}_{p}_{qc}_{hh}"
                        )
                        jmax = 4 if qc == 0 else NQT
                        for j in range(jmax):
                            ex = state[b][f"ex_{p}_{j}"]
                            n = S - 128 * j
                            lo = max(512 * qc, 128 * j) - 128 * j
                            hi = 512 * (qc + 1) - 128 * j
                            nc.tensor.matmul(
                                po[:, lo + 128 * j - 512 * qc : 512],
                                vxs[b][j][
                                    :, 128 * (2 * p + hh) : 128 * (2 * p + hh + 1)
                                ],
                                ex[:, n * hh + lo : n * hh + hi],
                                start=(j == 0),
                                stop=(j == jmax - 1),
                            )
                        pos[hh] = po
                    for hh in range(2):
                        po = pos[hh]
                        rcp = wk.tile([64, 512], F32, tag="rcp",
                                      name=f"rcp_{b}_{p}_{qc}_{hh}")
                        nc.vector.reciprocal(rcp, po[64:128, :])
                        nc.vector.tensor_mul(
                            ots[p][64 * hh : 64 * hh + 64,
                                   512 * qc : 512 * (qc + 1)],
                            po[0:64, :],
                            rcp,
                        )

            def proj_chunk(b, st):
                s0 = b * S
                ots = state[b]["ots"]
                pp = ps.tile([128, 512], F32, tag="ps", name=f"pp_{b}_{st}")
                for pt in range(NET):
                    nc.tensor.matmul(
                        pp,
                        ots[pt][:, 128 * st : 128 * (st + 1)],
                        wps[pt],
                        start=(pt == 0),
                        stop=False,
                    )
                nc.tensor.matmul(pp, ones1, brow_t, start=False, stop=True)
                ob = wk.tile([128, E], F32, tag="ob", name=f"ob_{b}_{st}")
                nc.vector.tensor_copy(ob, pp)
                nc.sync.dma_start(
                    out=out[s0 + 128 * st : s0 + 128 * (st + 1), :], in_=ob
                )

            def init_batch(b):
                state[b] = {
                    "qkts": [
                        qkp.tile([128, S], F16, tag=f"qkT{m}", name=f"qkT{m}_{b}")
                        for m in range(8)
                    ],
                    "ots": [
                        otp.tile([128, S], F16, tag=f"oT{p}", name=f"oT{p}_{b}")
                        for p in range(4)
                    ],
                }

            # ---- emission schedule ----
            init_batch(0)
            init_batch(1)
            # batch 0 qkv dense
            for m in range(8):
                for sc in range(2):
                    qkv_qk_chunk(0, m, sc)
            for st in range(NQT):
                qkv_v_chunk(0, st)
            # batch 0 pairs, with batch 1 qkv as PE filler
            fillers = [("qk", m, sc) for m in range(8) for sc in range(2)]
            fillers += [("v", st) for st in range(NQT)]
            fi = 0

            def emit_filler(k):
                nonlocal fi
                for _ in range(k):
                    if fi >= len(fillers):
                        return
                    f = fillers[fi]
                    fi += 1
                    if f[0] == "qk":
                        qkv_qk_chunk(1, f[1], f[2])
                    else:
                        qkv_v_chunk(1, f[1])

            for p in range(4):
                for j in range(NQT):
                    scores_exp(0, p, j)
                    if j % 3 == 2:
                        emit_filler(1)
                if p > 0:
                    attnv_norm(0, p - 1)
                emit_filler(2)
            attnv_norm(0, 3)
            emit_filler(len(fillers))
            # batch 1 pairs, with batch 0 proj as filler
            pfill = list(range(NQT))
            pi = 0
            for p in range(4):
                for j in range(NQT):
                    scores_exp(1, p, j)
                if p > 0:
                    attnv_norm(1, p - 1)
                if pi < NQT:
                    proj_chunk(0, pfill[pi])
                    proj_chunk(0, pfill[pi + 1])
                    pi += 2
            attnv_norm(1, 3)
            while pi < NQT:
                proj_chunk(0, pfill[pi])
                pi += 1
            for st in range(NQT):
                proj_chunk(1, st)
    _split_multi_waits(nc)
    return nc


# ---------------------------------------------------------------------------
# Host orchestration
# ---------------------------------------------------------------------------

_CACHE = {}


def _get(name, builder):
    if name not in _CACHE:
        _CACHE[name] = builder()
    return _CACHE[name]


def _run_with_retry(nc, in_maps, trace=False, tries=3):
    import time as _time

    last = None
    for attempt in range(tries):
        try:
            return run_bass_kernel_spmd(
                nc, in_maps, core_ids=list(range(NCORES)), trace=trace
            )
        except Exception as e:  # transient NRT_EXEC_UNIT_UNRECOVERABLE etc.
            last = e
            _time.sleep(2.0 * (attempt + 1))
    raise last


def _silu(v):
    return v / (1.0 + np.exp(-v))


def _pow2_scale(maxv, target=224.0):
    """Largest power of two s with maxv * s <= ~target (fp8e4 max 240)."""
    if maxv <= 0:
        return 1.0
    return float(2.0 ** np.floor(np.log2(target / maxv)))


def _to_fp8(x32, scale):
    import ml_dtypes

    return np.clip(x32 * scale, -240.0, 240.0).astype(ml_dtypes.float8_e4m3)


def kernel(
    time_embed,
    x,
    lin1_w,
    lin1_b,
    lin2_w,
    lin2_b,
    fW_attn_w,
    fW_attn_b,
    fb_attn,
    fW_proj_w,
    fW_proj_b,
    fb_proj,
    _trace=False,
    _times=None,
):
    f64 = np.float64
    # ---- host: time-embedding MLP ----
    t1 = _silu(time_embed.astype(f64) @ lin1_w.astype(f64) + lin1_b.astype(f64))
    t = t1 @ lin2_w.astype(f64) + lin2_b.astype(f64)   # [128]
    t16 = t.astype(np.float16)

    # ---- launch 1: W generation (fp8 LDW-path) ----
    nc_gen = _get("gen", build_gen)
    t32 = t.astype(np.float32)
    s_t = _pow2_scale(np.abs(t32).max())
    t_hi8 = _to_fp8(t32, s_t)
    t_resid = t32 * s_t - t_hi8.astype(np.float32)
    t_lo8 = _to_fp8(t_resid, 16.0)  # extra 4 mantissa bits
    tv_in = np.ascontiguousarray(np.stack(
        [t_hi8, t_lo8], axis=1))  # [128, 2] fp8

    fwa_flat = fW_attn_w.reshape(TEMBED, E * J3).astype(np.float32)
    fwp_flat = fW_proj_w.reshape(TEMBED, E * E).astype(np.float32)
    s_wa = _pow2_scale(np.abs(fwa_flat).max())
    s_wp = _pow2_scale(np.abs(fwp_flat).max())
    fw8 = np.concatenate(
        [_to_fp8(fwa_flat, s_wa), _to_fp8(fwp_flat, s_wp)], axis=1
    )  # [128, 1048576] fp8; cores 0-5 pure attn, 6-7 pure proj
    in_maps = []
    for c in range(NCORES):
        in_maps.append(
            {
                "tv": tv_in,
                "fw": fw8[:, GTOT * c : GTOT * (c + 1)],
            }
        )
    res1 = _run_with_retry(nc_gen, in_maps, trace=_trace)
    if _times is not None:
        _times.append(res1.exec_time_ns)

    slabs = []
    for c in range(NCORES):
        gv = res1.results[c]["g"]  # [128, 2048] f32, cols (hi, lo) pairs
        comb = gv[:, 0::2] + gv[:, 1::2] * (1.0 / 16.0)  # [128, 1024]
        s_w = s_wa if c < 6 else s_wp
        slabs.append(comb.T.reshape(-1) / (s_t * s_w))
    flat = np.concatenate(slabs)  # [1048576]
    Wa = flat[: E * J3].reshape(E, J3)
    Wp = flat[E * J3 :].reshape(E, E)
    Wa = Wa + fW_attn_b.reshape(E, J3)
    Wp = Wp + fW_proj_b.reshape(E, E)
    Wa[:, :512] *= 0.125  # fold 1/sqrt(D) into q columns

    # ---- host: biases ----
    b_attn = (t @ fb_attn.astype(f64).reshape(TEMBED, J3)).astype(np.float32)
    bqk_host = b_attn[:1024].copy()
    bqk_host[:512] *= 0.125
    bqk_in = np.ascontiguousarray(bqk_host.reshape(8, 128).T)
    b_v = b_attn[1024:]
    b_proj = (t @ fb_proj.astype(f64)).astype(np.float32)
    brow = (b_v.astype(f64) @ Wp.astype(f64) + b_proj).astype(np.float16)
    brow_in = np.ascontiguousarray(brow[None, :])
    mask_in = np.triu(np.ones((128, 128), dtype=np.float16))
    ones_in = np.ones((128, 128), dtype=np.float16)
    Wa16 = Wa.astype(np.float16)
    Wp16 = Wp.astype(np.float16)

    # ---- launch 2: attention ----
    if os.environ.get("ATTN_V1", "0") == "1":
        nc_attn = _get("attn", build_attn)
    else:
        nc_attn = _get("attn2", build_attn2)
    in_maps = []
    for c in range(NCORES):
        xt_c = np.ascontiguousarray(
            x[BPC * c : BPC * (c + 1)].reshape(S2, E).T
        )
        in_maps.append(
            {
                "xt": xt_c.astype(np.float16),
                "wa": Wa16,
                "wp": Wp16,
                "bqk": bqk_in,
                "brow": brow_in,
                "mask": mask_in,
                "ones": ones_in,
            }
        )
    res2 = _run_with_retry(nc_attn, in_maps, trace=_trace)
    if _times is not None:
        _times.append(res2.exec_time_ns)

    out = np.empty((B, S, E), dtype=np.float32)
    for c in range(NCORES):
        out[BPC * c : BPC * (c + 1)] = res2.results[c]["out"].reshape(BPC, S, E)
    return out



# revision 24
# speedup vs baseline: 1.1332x; 1.1332x over previous
"""Hypernetwork causal attention (nn_Attention_87926570484382) on 8 TRN2 cores.

Strategy (two launches, batch-sharded attention, gen-sharded hypernet):
  host   : time-embedding MLP -> t [128]; bias assembly; q-scale folding.
  launch1: each core streams 1/8 of fW_attn_w / fW_proj_w through the PE
           (stationary = t replicated across 128 cols, f32r) producing its
           slice of W_attn / W_proj.
  host   : gather W slices, add biases, fold 1/sqrt(D) into the q columns.
  launch2: each core runs full attention for 2 of the 16 batches:
           qk^T = W_attn^T-tiles @ x^T   (heads on partitions)
           scores^T[k,q] per (b,h), exp (no max-sub; causal mask as
           multiply on diagonal tiles), attn@V via ones-extended V (softmax
           denominators land in psum rows 64..127), per-head normalize,
           proj with bias via K=1 matmul, out[s,e].

All matmuls run in float32r (TF32-like, 1 cycle/row); rel err ~2e-4.
"""

import os
import sys

import numpy as np

# ---------------------------------------------------------------------------
# Environment shims (must precede concourse imports in fresh environments)
# ---------------------------------------------------------------------------


def _ensure_axon_hooks():
    """Provide antenv.axon_hooks if the installed antenv lacks it (needed
    only when tracing; harmless otherwise)."""
    try:
        import antenv.axon_hooks  # noqa: F401
        return
    except ImportError:
        pass
    try:
        import antenv
    except ImportError:
        return
    import contextlib
    import ctypes
    import types

    mod = types.ModuleType("antenv.axon_hooks")
    mod._HOOK = None
    mod._TRIED = False

    def set_axon_ntff_profile_hook(hook):
        mod._HOOK = hook

    def _build(so_path):
        lib = ctypes.CDLL(so_path)
        if not hasattr(lib, "axon_start_nrt_profile"):
            return None
        lib.axon_start_nrt_profile.argtypes = [
            ctypes.POINTER(ctypes.c_int64),
            ctypes.c_size_t,
        ]
        lib.axon_start_nrt_profile.restype = ctypes.c_int64
        lib.axon_stop_nrt_profile.argtypes = [ctypes.c_char_p]
        lib.axon_stop_nrt_profile.restype = ctypes.c_int64

        @contextlib.contextmanager
        def _hook(output_dir, device_ids):
            import jax

            jax.devices()
            if device_ids:
                ids = (ctypes.c_int64 * len(device_ids))(*device_ids)
                rc = lib.axon_start_nrt_profile(ids, len(device_ids))
            else:
                rc = lib.axon_start_nrt_profile(None, 0)
            if rc != 0:
                raise RuntimeError(f"axon_start_nrt_profile rc={rc}")
            try:
                yield
            finally:
                n = lib.axon_stop_nrt_profile(str(output_dir).encode())
                print(f"profile: {n} file(s) -> {output_dir}", file=sys.stderr)

        return _hook

    def get_axon_ntff_profile_hook():
        if mod._HOOK is None and not mod._TRIED:
            mod._TRIED = True
            p = "/opt/axon/libaxon_pjrt.so"
            if os.path.exists(p):
                try:
                    mod._HOOK = _build(p)
                except OSError:
                    mod._HOOK = None
        return mod._HOOK

    mod.set_axon_ntff_profile_hook = set_axon_ntff_profile_hook
    mod.get_axon_ntff_profile_hook = get_axon_ntff_profile_hook
    sys.modules["antenv.axon_hooks"] = mod
    antenv.axon_hooks = mod


_ensure_axon_hooks()

import concourse.bass as bass  # noqa: E402
import concourse.mybir as mybir  # noqa: E402
from concourse import tile as _tile  # noqa: E402
from concourse.tile import TileContext, add_dep_helper  # noqa: E402
from concourse.vector_clock import ScopedClock  # noqa: E402
from concourse.bass_utils import run_bass_kernel_spmd  # noqa: E402

F32 = mybir.dt.float32
F32R = mybir.dt.float32r
F16 = mybir.dt.float16
FP8 = mybir.dt.float8e4

# problem constants (hardcoded per harness contract)
SIN_DIM, TEMBED = 64, 128
E, H, D = 512, 8, 64
B, S = 16, 1024
NCORES = 8
SKIP_NR = os.environ.get("SKIP_NR", "1") == "1"
BPC = B // NCORES          # batches per core
S2 = BPC * S               # 2048 rows per core
J3 = 3 * H * D             # 1536
EJ_A = E * J3 // NCORES    # 98304  fW_attn slab cols per core
EJ_P = E * E // NCORES     # 32768  fW_proj slab cols per core
GTOT = (E * J3 + E * E) // NCORES   # 131072 slab cols per core (concat split)
GT_TILES = GTOT // 128              # 1024 stationary tiles per core
NT = 2                              # t columns per matmul (hi, lo fp8)
GROUP_TILES = 512 // NT             # tiles per psum bank

# ---------------------------------------------------------------------------
# Tile framework workarounds: this walrus accepts at most ONE semaphore wait
# and one update per instruction.
# ---------------------------------------------------------------------------

_NOP_CTR = [0]


def _patched_drain_and_barrier(self, tick_clock, wait_clock):
    carrier = self.nc.sync.nop(nofuse=True)
    wait_clock.add_sem_waits(
        carrier.ins, ScopedClock({None: tick_clock.global_clock})
    )
    si = carrier.ins.sync_info
    waits = list(si.on_wait) if si and si.on_wait else []
    if len(waits) > 1:
        carrier.ins.sync_info = mybir.SyncInfo(
            on_wait=waits[:1],
            on_update=list(si.on_update) if si and si.on_update else [],
        )
        for w in waits[1:]:
            extra = self.nc.sync.nop(nofuse=True)
            extra.ins.sync_info = mybir.SyncInfo(on_wait=[w], on_update=[])
    self.nc.sync.drain()
    self.nc.all_engine_barrier()
    assert self.sems is not None
    popped = self.nc._tile_sem_poison_stack.pop()
    assert popped is self._sem_poison
    self.nc.clear_and_free_semaphores(list(self.sems.allocated().values()))
    self.nc.all_engine_barrier()


_tile.TileContext._drain_and_barrier = _patched_drain_and_barrier


def _split_multi_waits(nc):
    for f in nc.m.functions:
        for blk in f.blocks:
            out = []
            changed = False
            for inst in blk.instructions:
                si = inst.sync_info
                waits = list(si.on_wait) if si and si.on_wait else []
                updates = list(si.on_update) if si and si.on_update else []
                is_dma = "DMA" in type(inst).__name__
                if len(waits) > 1:
                    changed = True
                    for w in waits[:-1]:
                        _NOP_CTR[0] += 1
                        nop = mybir.InstNoOp(
                            name=f"wsplit_{_NOP_CTR[0]}", ins=[], outs=[]
                        )
                        nop.engine = inst.engine
                        nop.sync_info = mybir.SyncInfo(on_wait=[w], on_update=[])
                        out.append(nop)
                    waits = [waits[-1]]
                    inst.sync_info = mybir.SyncInfo(
                        on_wait=waits, on_update=updates
                    )
                out.append(inst)
                if len(updates) > 1:
                    if is_dma:
                        raise AssertionError(
                            f"DMA {inst.name} has {len(updates)} updates"
                        )
                    changed = True
                    inst.sync_info = mybir.SyncInfo(
                        on_wait=waits, on_update=[updates[0]]
                    )
                    for u in updates[1:]:
                        _NOP_CTR[0] += 1
                        nop = mybir.InstNoOp(
                            name=f"usplit_{_NOP_CTR[0]}", ins=[], outs=[]
                        )
                        nop.engine = inst.engine
                        nop.sync_info = mybir.SyncInfo(on_wait=[], on_update=[u])
                        out.append(nop)
            if changed:
                blk.instructions = out
    return nc


# ---------------------------------------------------------------------------
# Launch 1: hypernetwork weight generation
# ---------------------------------------------------------------------------


def build_gen():
    """Per core: W_slab[c] = sum_k t[k] * fW_slab[k, c] for a 1/8 slice of
    the concatenated (fW_attn_w | fW_proj_w) column space.

    Flipped matmul orientation: the fp8 fW tile [128k, 128j] is the
    STATIONARY operand (FWL-accelerated LDWEIGHTS carries the bandwidth)
    and t is the moving operand ([128, 2]: hi/lo fp8 split), so each
    matmul emits a compact [128, 2] psum column pair -- no replicated
    output, copies shrink from 64 MB to 1 MB per core.  Host combines
    hi + lo/16 and rescales."""
    nc = bass.Bass()
    fw = nc.dram_tensor("fw", [TEMBED, GTOT], FP8, kind="ExternalInput")
    tv = nc.dram_tensor("tv", [TEMBED, NT], FP8, kind="ExternalInput")
    g = nc.dram_tensor("g", [128, GT_TILES * NT], F32, kind="ExternalOutput")

    GCHUNK = 8192           # fW cols per DMA chunk (1 MB fp8)
    TPC = GCHUNK // 128     # 64 stationary tiles per chunk

    with TileContext(nc) as tc:
        with (
            tc.tile_pool(name="cst", bufs=1) as cst,
            tc.tile_pool(name="sb", bufs=5) as sb,
            tc.tile_pool(name="gs", bufs=2) as gsp,
            tc.tile_pool(name="ps", bufs=2, space="PSUM") as ps,
        ):
            tt = cst.tile([TEMBED, NT], FP8)
            nc.sync.dma_start(out=tt, in_=tv[:, :])

            pg = None
            for chunk in range(GTOT // GCHUNK):
                ft = sb.tile([TEMBED, GCHUNK], FP8, tag="fw", name=f"fw{chunk}")
                # alternate the two HWDGE rings so fixed DMA latencies overlap
                eng = nc.sync if chunk % 2 == 0 else nc.scalar
                eng.dma_start(
                    out=ft, in_=fw[:, chunk * GCHUNK : (chunk + 1) * GCHUNK]
                )
                for i in range(TPC):
                    c = chunk * TPC + i
                    grp, off = divmod(c, GROUP_TILES)
                    if off == 0:
                        pg = ps.tile([128, 512], F32, tag="pg", name=f"pg{grp}")
                    nc.tensor.matmul(
                        pg[:, NT * off : NT * (off + 1)],
                        ft[:, 128 * i : 128 * (i + 1)],
                        tt,
                        start=True,
                        stop=True,
                    )
                    if off == GROUP_TILES - 1:
                        gt = gsp.tile([128, 512], F32, tag="gs", name=f"gs{grp}")
                        nc.vector.tensor_copy(gt, pg)
                        # outputs ride SWDGE so they never head-of-line
                        # block the input chunk stream
                        nc.gpsimd.dma_start(
                            out=g[:, 512 * grp : 512 * (grp + 1)], in_=gt
                        )
    _split_multi_waits(nc)
    return nc


# ---------------------------------------------------------------------------
# Launch 2: attention for 2 batches per core
# ---------------------------------------------------------------------------


def _act_recip_seed(nc, out, in_):
    eng = nc.scalar
    imm = lambda v: mybir.ImmediateValue(dtype=mybir.dt.float32, value=v)
    return eng.add_instruction(
        mybir.InstActivation(
            name=eng.bass.get_next_instruction_name(),
            func=mybir.ActivationFunctionType.Reciprocal,
            ins=[eng.lower_ap(in_), imm(0.0), imm(1.0), imm(0.0)],
            outs=[eng.lower_ap(out)],
        )
    )


def build_attn():
    nc = bass.Bass()
    xt = nc.dram_tensor("xt", [E, S2], F16, kind="ExternalInput")
    wa = nc.dram_tensor("wa", [E, J3], F16, kind="ExternalInput")
    wp = nc.dram_tensor("wp", [E, E], F16, kind="ExternalInput")
    bqk = nc.dram_tensor("bqk", [128, 8], F32, kind="ExternalInput")
    brow = nc.dram_tensor("brow", [1, E], F16, kind="ExternalInput")
    mask = nc.dram_tensor("mask", [128, 128], F16, kind="ExternalInput")
    ones = nc.dram_tensor("ones", [128, 128], F16, kind="ExternalInput")
    out = nc.dram_tensor("out", [S2, E], F32, kind="ExternalOutput")

    NQT = S // 128           # 8 q/k tiles per batch
    NET = E // 128           # 4 e tiles

    with TileContext(nc) as tc:
        with (
            tc.tile_pool(name="cst", bufs=1) as cst,
            tc.tile_pool(name="qk", bufs=1) as qkp,
            tc.tile_pool(name="vx", bufs=1) as vxp,
            tc.tile_pool(name="ot", bufs=1) as otp,
            tc.tile_pool(name="ou", bufs=1) as oup,
            tc.tile_pool(name="wk", bufs=3) as wk,
            tc.tile_pool(name="ex", bufs=4) as exp_pool,
            tc.tile_pool(name="ps", bufs=3, space="PSUM") as ps,
            tc.tile_pool(name="po", bufs=5, space="PSUM") as pop,
        ):
            # resident inputs
            xts = []
            was = []
            wps = []
            for et in range(NET):
                t_ = cst.tile([128, S2], F16, tag=f"xt{et}")
                nc.sync.dma_start(out=t_, in_=xt[128 * et : 128 * (et + 1), :])
                xts.append(t_)
                t_ = cst.tile([128, J3], F16, tag=f"wa{et}")
                nc.sync.dma_start(out=t_, in_=wa[128 * et : 128 * (et + 1), :])
                was.append(t_)
                t_ = cst.tile([128, E], F16, tag=f"wp{et}")
                nc.sync.dma_start(out=t_, in_=wp[128 * et : 128 * (et + 1), :])
                wps.append(t_)
            bqk_t = cst.tile([128, 8], F32)
            nc.sync.dma_start(out=bqk_t, in_=bqk[:, :])
            brow_t = cst.tile([1, E], F16)
            nc.sync.dma_start(out=brow_t, in_=brow[:, :])
            mask_t = cst.tile([128, 128], F16)
            nc.sync.dma_start(out=mask_t, in_=mask[:, :])
            ones1 = cst.tile([1, 128], F16)
            nc.sync.dma_start(out=ones1, in_=ones[0:1, :])
            # persistent v_ext tiles: [128 k-rows, 8h * (64 v | 64 ones)];
            # ones columns are constant, v columns rewritten per batch
            vxs = []
            for st in range(S // 128):
                v_ = vxp.tile([128, 8 * 128], F16, tag=f"vx{st}", name=f"vx{st}")
                for h in range(H):
                    nc.sync.dma_start(
                        out=v_[:, 128 * h + 64 : 128 * (h + 1)],
                        in_=ones[:, 0:64],
                    )
                vxs.append(v_)

            state = {}

            def qkv_phase(b):
                s0 = b * S
                qkts = []
                for m in range(8):
                    qkts.append(qkp.tile([128, S], F16, tag=f"qkT{m}", name=f"qkT{m}_{b}"))
                for m in range(8):
                    for sc in range(S // 512):
                        pq = ps.tile([128, 512], F32, tag="ps", name=f"pq_{b}_{m}_{sc}")
                        for et in range(NET):
                            nc.tensor.matmul(
                                pq,
                                was[et][:, 128 * m : 128 * (m + 1)],
                                xts[et][:, s0 + 512 * sc : s0 + 512 * (sc + 1)],
                                start=(et == 0),
                                stop=(et == NET - 1),
                            )
                        nc.vector.tensor_scalar_add(
                            qkts[m][:, 512 * sc : 512 * (sc + 1)],
                            pq,
                            bqk_t[:, m : m + 1],
                        )
                for st in range(NQT):
                    pv = ps.tile([128, 512], F32, tag="ps", name=f"pv_{b}_{st}")
                    for et in range(NET):
                        nc.tensor.matmul(
                            pv,
                            xts[et][:, s0 + 128 * st : s0 + 128 * (st + 1)],
                            was[et][:, 1024:1536],
                            start=(et == 0),
                            stop=(et == NET - 1),
                        )
                    for h in range(H):
                        if h % 2 == 0:
                            nc.vector.tensor_copy(
                                vxs[st][:, 128 * h : 128 * h + 64],
                                pv[:, 64 * h : 64 * (h + 1)],
                            )
                        else:
                            nc.scalar.copy(
                                vxs[st][:, 128 * h : 128 * h + 64],
                                pv[:, 64 * h : 64 * (h + 1)],
                            )
                state[b] = {"qkts": qkts}

            def attn_phase(b):
                st_ = state[b]
                qkts = st_["qkts"]
                ots = [
                    otp.tile([128, S], F16, tag=f"oT{ht}", name=f"oT{ht}_{b}")
                    for ht in range(NET)
                ]
                osm = [
                    oup.tile([128, S], F32, tag=f"ou{h}", name=f"ou{h}_{b}")
                    for h in range(H)
                ]
                st_.update(ots=ots, osm=osm, exps=[])
                # sandwich: this batch's exps must follow the previous
                # batch's reciprocal seeds (ACT table thrash avoidance)
                prev = state.get(b - 1)
                if prev and prev.get("seeds"):
                    st_["first_exp_dep"] = prev["seeds"][-1]
                for h in range(H):
                    kt_tile = qkts[4 + h // 2]
                    qt_tile = qkts[h // 2]
                    prow = 64 * (h % 2)
                    po0 = pop.tile([128, 512], F32, tag="po", name=f"po0_{b}_{h}")
                    po1 = pop.tile([128, 512], F32, tag="po", name=f"po1_{b}_{h}")
                    pos = (po0, po1)

                    def scores_exp(j):
                        expt = exp_pool.tile(
                            [128, S], F16, tag="expt", name=f"expt_{b}_{h}_{j}"
                        )
                        for qc in range(2):
                            c0 = max(512 * qc, 128 * j)
                            c1 = 512 * (qc + 1)
                            if c0 >= c1:
                                continue
                            pst = ps.tile(
                                [128, 512], F32, tag="ps", name=f"pst_{b}_{h}_{j}_{qc}"
                            )
                            nc.tensor.matmul(
                                pst[:, 0 : c1 - c0],
                                kt_tile[prow : prow + 64, 128 * j : 128 * (j + 1)],
                                qt_tile[prow : prow + 64, c0:c1],
                                start=True,
                                stop=True,
                            )
                            ei = nc.scalar.activation(
                                expt[:, c0:c1],
                                pst[:, 0 : c1 - c0],
                                func=mybir.ActivationFunctionType.Exp,
                            )
                            st_["exps"].append(ei.ins)
                        nc.vector.tensor_mul(
                            expt[:, 128 * j : 128 * (j + 1)],
                            expt[:, 128 * j : 128 * (j + 1)],
                            mask_t,
                        )
                        return expt

                    expts = {0: scores_exp(0), 1: scores_exp(1)}
                    for j in range(NQT):
                        if j + 2 < NQT:
                            expts[j + 2] = scores_exp(j + 2)
                        expt = expts.pop(j)
                        for qc in range(2):
                            c0 = max(512 * qc, 128 * j)
                            c1 = 512 * (qc + 1)
                            if c0 >= c1:
                                continue
                            nc.tensor.matmul(
                                pos[qc][:, c0 - 512 * qc : c1 - 512 * qc],
                                vxs[j][:, 128 * h : 128 * (h + 1)],
                                expt[:, c0:c1],
                                start=(j == 0),
                                stop=(j == NQT - 1 if qc == 1 else j == 3),
                            )
                    for qc in range(2):
                        nc.vector.tensor_copy(
                            osm[h][:, 512 * qc : 512 * (qc + 1)],
                            pos[qc][:, :],
                        )

            def norm_phase(b):
                st_ = state[b]
                ots, osm = st_["ots"], st_["osm"]
                y0s = {}
                st_["seeds"] = []
                for h in range(H):
                    y0s[h] = wk.tile([64, S], F32, tag="y0", name=f"y0_{b}_{h}")
                    si = _act_recip_seed(nc, y0s[h], osm[h][64:128, :])
                    st_["seeds"].append(si.ins)
                for h in range(H):
                    prow = 64 * (h % 2)
                    if SKIP_NR:
                        rec = y0s[h]
                    else:
                        tvar = wk.tile([64, S], F32, tag="tvar", name=f"tv_{b}_{h}")
                        nc.vector.tensor_mul(tvar, osm[h][64:128, :], y0s[h])
                        nc.vector.tensor_scalar(
                            out=tvar, in0=tvar, scalar1=-1.0, scalar2=2.0,
                            op0=mybir.AluOpType.mult, op1=mybir.AluOpType.add,
                        )
                        rec = wk.tile([64, S], F32, tag="rec", name=f"rec_{b}_{h}")
                        nc.vector.tensor_mul(rec, tvar, y0s[h])
                    nc.vector.tensor_mul(
                        ots[h // 2][prow : prow + 64, :], osm[h][0:64, :], rec
                    )

            def proj_phase(b, st_lo=0, st_hi=NQT):
                s0 = b * S
                ots = state[b]["ots"]
                for st in range(st_lo, st_hi):
                    pp = ps.tile([128, 512], F32, tag="ps", name=f"pp_{b}_{st}")
                    for ht in range(NET):
                        nc.tensor.matmul(
                            pp,
                            ots[ht][:, 128 * st : 128 * (st + 1)],
                            wps[ht],
                            start=(ht == 0),
                            stop=False,
                        )
                    nc.tensor.matmul(
                        pp, ones1, brow_t, start=False, stop=True
                    )
                    ob = wk.tile([128, E], F32, tag="ob", name=f"ob_{b}_{st}")
                    nc.vector.tensor_copy(ob, pp)
                    nc.sync.dma_start(
                        out=out[s0 + 128 * st : s0 + 128 * (st + 1), :], in_=ob
                    )

            # batch-level software pipeline: norm(b)/proj(b) overlap the
            # next batch's dense PE phases
            qkv_phase(0)
            attn_phase(0)
            qkv_phase(1)
            norm_phase(0)
            attn_phase(1)
            proj_phase(0, 0, 4)
            norm_phase(1)
            proj_phase(0, 4, NQT)
            proj_phase(1)
    _split_multi_waits(nc)
    return nc


# ---------------------------------------------------------------------------
# Launch 2 (v2): attention, restructured for HAM warmth + merged exp calls
# ---------------------------------------------------------------------------


def build_attn2():
    """Per core: full causal attention for 2 batches.

    vs v1: (1) head-pair score matmuls issued adjacently so the K=64 row
    tiles (partitions 0-63 / 64-127) run concurrently on the PE;
    (2) scores for a (pair, j) land in one 4-bank psum tile and are
    exponentiated by ONE ACT call via a [128, 2, N] access pattern
    (6 calls/pair instead of 28); (3) softmax normalize via DVE
    reciprocal_approx_fast straight out of psum (no Reciprocal table
    thrash, no replicated copies); (4) qkv/proj chunks of the
    neighbouring batch are interleaved into the pair loop to fill the
    PE gaps left by the exp dependency chain."""
    nc = bass.Bass()
    xt = nc.dram_tensor("xt", [E, S2], F16, kind="ExternalInput")
    wa = nc.dram_tensor("wa", [E, J3], F16, kind="ExternalInput")
    wp = nc.dram_tensor("wp", [E, E], F16, kind="ExternalInput")
    bqk = nc.dram_tensor("bqk", [128, 8], F32, kind="ExternalInput")
    brow = nc.dram_tensor("brow", [1, E], F16, kind="ExternalInput")
    mask = nc.dram_tensor("mask", [128, 128], F16, kind="ExternalInput")
    ones = nc.dram_tensor("ones", [128, 128], F16, kind="ExternalInput")
    out = nc.dram_tensor("out", [S2, E], F32, kind="ExternalOutput")

    NQT = S // 128           # 8 k-tiles (and q-tiles) per batch
    NET = E // 128           # 4 e-tiles

    with TileContext(nc) as tc:
        with (
            tc.tile_pool(name="cst", bufs=1) as cst,
            tc.tile_pool(name="qk", bufs=1) as qkp,
            tc.tile_pool(name="vx", bufs=1) as vxp,
            tc.tile_pool(name="ot", bufs=1) as otp,
            tc.tile_pool(name="ex", bufs=2) as exp_pool,
            tc.tile_pool(name="os", bufs=1) as osp,
            tc.tile_pool(name="wk", bufs=3) as wk,
            tc.tile_pool(name="sp", bufs=1, space="PSUM") as spp,
            tc.tile_pool(name="po", bufs=2, space="PSUM") as pop,
            tc.tile_pool(name="ps", bufs=2, space="PSUM") as ps,
        ):
            # ---- streamed input residency (few big DMAs; wa first so the
            # first qk matmuls can start as soon as x batch 0 lands) ----
            xts = [
                cst.tile([128, S2], F16, tag=f"xt{et}", name=f"xts{et}")
                for et in range(NET)
            ]
            was = [
                cst.tile([128, J3], F16, tag=f"wa{et}", name=f"was{et}")
                for et in range(NET)
            ]
            wps = [
                cst.tile([128, E], F16, tag=f"wp{et}", name=f"wps{et}")
                for et in range(NET)
            ]
            bqk_t = cst.tile([128, 8], F32)
            nc.sync.dma_start(out=bqk_t, in_=bqk[:, :])
            for et in range(NET):
                nc.scalar.dma_start(
                    out=was[et], in_=wa[128 * et : 128 * (et + 1), :]
                )
            for et in range(NET):
                nc.sync.dma_start(
                    out=xts[et], in_=xt[128 * et : 128 * (et + 1), :]
                )
            mask_t = cst.tile([128, 128], F16)
            nc.sync.dma_start(out=mask_t, in_=mask[:, :])
            ones1 = cst.tile([1, 128], F16)
            nc.sync.dma_start(out=ones1, in_=ones[0:1, :])
            brow_t = cst.tile([1, E], F16)
            nc.sync.dma_start(out=brow_t, in_=brow[:, :])
            for et in range(NET):
                nc.scalar.dma_start(
                    out=wps[et], in_=wp[128 * et : 128 * (et + 1), :]
                )
            # v_ext tiles: [128 k, 8h * (64 v | 64 ones)]; per batch so the
            # next batch's v generation never WAR-blocks on this batch's
            # attn@V reads.  ones cols come from a gpsimd memset, v cols
            # are overwritten by the v-generation copy.
            vxs = {}
            for b in range(BPC):
                vxs[b] = []
                for st in range(NQT):
                    v_ = vxp.tile(
                        [128, 8 * 128], F16, tag=f"vx{st}_{b}", name=f"vx{st}_{b}"
                    )
                    nc.gpsimd.memset(v_, 1.0)
                    vxs[b].append(v_)

            state = {}

            # ---- phase pieces, emitted in interleaved order below ----

            def qkv_qk_chunk(b, m, sc):
                """qk^T[128 cols of m-chunk, 512 s] for batch b."""
                s0 = b * S
                qkts = state[b]["qkts"]
                pq = ps.tile([128, 512], F32, tag="ps", name=f"pq_{b}_{m}_{sc}")
                for et in range(NET):
                    nc.tensor.matmul(
                        pq,
                        was[et][:, 128 * m : 128 * (m + 1)],
                        xts[et][:, s0 + 512 * sc : s0 + 512 * (sc + 1)],
                        start=(et == 0),
                        stop=(et == NET - 1),
                    )
                nc.vector.tensor_scalar_add(
                    qkts[m][:, 512 * sc : 512 * (sc + 1)],
                    pq,
                    bqk_t[:, m : m + 1],
                )

            def qkv_v_chunk(b, st):
                """v rows for s-tile st of batch b, scattered into vxs."""
                s0 = b * S
                pv = ps.tile([128, 512], F32, tag="ps", name=f"pv_{b}_{st}")
                for et in range(NET):
                    nc.tensor.matmul(
                        pv,
                        xts[et][:, s0 + 128 * st : s0 + 128 * (st + 1)],
                        was[et][:, 1024:1536],
                        start=(et == 0),
                        stop=(et == NET - 1),
                    )
                # one strided copy: [128, 8, 64] psum -> v cols of vxs[b][st]
                nc.vector.tensor_copy(
                    vxs[b][st][:, 0 : 8 * 128].rearrange(
                        "p (h t) -> p h t", h=8
                    )[:, :, 0:64],
                    pv.rearrange("p (h d) -> p h d", h=8),
                )

            def scores_exp(b, p, j):
                """Scores + exp for head pair p, k-tile j: two concurrent
                K=64 matmul chains into one 4-bank psum tile, one exp."""
                qkts = state[b]["qkts"]
                kt, qt = qkts[4 + p], qkts[p]
                c0 = 128 * j
                n = S - c0
                sp = spp.tile([128, 2048], F32, tag="spair", name=f"sp_{b}_{p}_{j}")
                for lo, hi in ((c0, 512), (max(512, c0), 1024)):
                    if lo >= hi:
                        continue
                    for hh in range(2):
                        nc.tensor.matmul(
                            sp[:, 1024 * hh + lo : 1024 * hh + hi],
                            kt[64 * hh : 64 * hh + 64, c0 : c0 + 128],
                            qt[64 * hh : 64 * hh + 64, lo:hi],
                            start=True,
                            stop=True,
                        )
                ex = exp_pool.tile(
                    [128, 2 * n], F16, tag=f"ex{j}", name=f"ex_{b}_{p}_{j}"
                )
                ei = nc.scalar.activation(
                    ex.rearrange("p (h q) -> p h q", h=2),
                    sp.rearrange("p (h q) -> p h q", h=2)[:, :, c0:S],
                    func=mybir.ActivationFunctionType.Exp,
                )
                state[b].setdefault("exps", []).append(ei.ins)
                # causal mask on the diagonal 128-col block of both heads
                # (gpsimd: SBUF-only op on an otherwise idle engine)
                nc.gpsimd.tensor_mul(
                    ex.rearrange("p (h q) -> p h q", h=2)[:, :, 0:128],
                    ex.rearrange("p (h q) -> p h q", h=2)[:, :, 0:128],
                    mask_t.unsqueeze(1).broadcast_to((128, 2, 128)),
                )
                state[b][f"ex_{p}_{j}"] = ex

            def attnv(b, p):
                """attn @ V_ext for pair p (2 q-passes); unnormalized o and
                replicated denominators staged to SBUF (f16) so psum frees
                immediately and the reciprocals can batch at batch end."""
                for qc in range(2):
                    for hh in range(2):
                        po = pop.tile(
                            [128, 512], F32, tag="po", name=f"po_{b}_{p}_{qc}_{hh}"
                        )
                        jmax = 4 if qc == 0 else NQT
                        for j in range(jmax):
                            ex = state[b][f"ex_{p}_{j}"]
                            n = S - 128 * j
                            lo = max(512 * qc, 128 * j) - 128 * j
                            hi = 512 * (qc + 1) - 128 * j
                            nc.tensor.matmul(
                                po[:, lo + 128 * j - 512 * qc : 512],
                                vxs[b][j][
                                    :, 128 * (2 * p + hh) : 128 * (2 * p + hh + 1)
                                ],
                                ex[:, n * hh + lo : n * hh + hi],
                                start=(j == 0),
                                stop=(j == jmax - 1),
                            )
                        nc.vector.tensor_copy(
                            state[b]["osm"][2 * p + hh][:, 512 * qc : 512 * (qc + 1)],
                            po,
                        )

            def norm(b):
                """Batched softmax normalization: one ACT Reciprocal window
                per batch (single table-set switch), then DVE muls."""
                ots, osm = state[b]["ots"], state[b]["osm"]
                rcps = {}
                for h in range(H):
                    rcp = wk.tile([64, S], F16, tag="rcp", name=f"rcp_{b}_{h}")
                    ri = _act_recip_seed(nc, rcp, osm[h][64:128, :])
                    state[b].setdefault("recips", []).append(ri.ins)
                    rcps[h] = rcp
                for h in range(H):
                    nc.vector.tensor_mul(
                        ots[h // 2][64 * (h % 2) : 64 * (h % 2) + 64, :],
                        osm[h][0:64, :],
                        rcps[h],
                    )

            def proj_chunk(b, st):
                s0 = b * S
                ots = state[b]["ots"]
                pp = ps.tile([128, 512], F32, tag="ps", name=f"pp_{b}_{st}")
                for pt in range(NET):
                    nc.tensor.matmul(
                        pp,
                        ots[pt][:, 128 * st : 128 * (st + 1)],
                        wps[pt],
                        start=(pt == 0),
                        stop=False,
                    )
                nc.tensor.matmul(pp, ones1, brow_t, start=False, stop=True)
                ob = wk.tile([128, E], F32, tag="ob", name=f"ob_{b}_{st}")
                nc.vector.tensor_copy(ob, pp)
                nc.sync.dma_start(
                    out=out[s0 + 128 * st : s0 + 128 * (st + 1), :], in_=ob
                )

            def init_batch(b):
                state[b] = {
                    "qkts": [
                        qkp.tile([128, S], F16, tag=f"qkT{m}", name=f"qkT{m}_{b}")
                        for m in range(8)
                    ],
                    "ots": [
                        otp.tile([128, S], F16, tag=f"oT{p}", name=f"oT{p}_{b}")
                        for p in range(4)
                    ],
                    "osm": [
                        osp.tile([128, S], F16, tag=f"osm{h}", name=f"osm{h}_{b}")
                        for h in range(H)
                    ],
                }

            # ---- emission schedule ----
            init_batch(0)
            init_batch(1)
            # batch 0 qkv dense
            for m in range(8):
                for sc in range(2):
                    qkv_qk_chunk(0, m, sc)
            for st in range(NQT):
                qkv_v_chunk(0, st)
            # batch 0 pairs, with batch 1 qkv as PE filler
            fillers = [("qk", m, sc) for m in range(8) for sc in range(2)]
            fillers += [("v", st) for st in range(NQT)]
            fi = 0

            def emit_filler(k):
                nonlocal fi
                for _ in range(k):
                    if fi >= len(fillers):
                        return
                    f = fillers[fi]
                    fi += 1
                    if f[0] == "qk":
                        qkv_qk_chunk(1, f[1], f[2])
                    else:
                        qkv_v_chunk(1, f[1])

            for p in range(4):
                for j in range(NQT):
                    scores_exp(0, p, j)
                    if j % 3 == 2:
                        emit_filler(1)
                if p > 0:
                    attnv(0, p - 1)
                emit_filler(2)
            attnv(0, 3)
            # batched reciprocal window for batch 0 (PE runs qkv(1) fillers)
            norm(0)
            emit_filler(len(fillers))
            # batch 1 pairs, with batch 0 proj as filler
            for p in range(4):
                for j in range(NQT):
                    scores_exp(1, p, j)
                if p > 0:
                    attnv(1, p - 1)
                proj_chunk(0, 2 * p)
                proj_chunk(0, 2 * p + 1)
            attnv(1, 3)
            norm(1)
            for st in range(NQT):
                proj_chunk(1, st)
            # keep ACT's table-set switches to 3: all of batch 1's exps run
            # after batch 0's reciprocal window
            for ei in state[1]["exps"]:
                for ri in state[0]["recips"]:
                    add_dep_helper(ei, ri, sync=False,
                                   reason="act table-set batching")
                break
    _split_multi_waits(nc)
    return nc


# ---------------------------------------------------------------------------
# Host orchestration
# ---------------------------------------------------------------------------

_CACHE = {}


def _get(name, builder):
    if name not in _CACHE:
        _CACHE[name] = builder()
    return _CACHE[name]


def _run_with_retry(nc, in_maps, trace=False, tries=3):
    import time as _time

    last = None
    for attempt in range(tries):
        try:
            return run_bass_kernel_spmd(
                nc, in_maps, core_ids=list(range(NCORES)), trace=trace
            )
        except Exception as e:  # transient NRT_EXEC_UNIT_UNRECOVERABLE etc.
            last = e
            _time.sleep(2.0 * (attempt + 1))
    raise last


def _silu(v):
    return v / (1.0 + np.exp(-v))


def _pow2_scale(maxv, target=224.0):
    """Largest power of two s with maxv * s <= ~target (fp8e4 max 240)."""
    if maxv <= 0:
        return 1.0
    return float(2.0 ** np.floor(np.log2(target / maxv)))


def _to_fp8(x32, scale):
    import ml_dtypes

    return np.clip(x32 * scale, -240.0, 240.0).astype(ml_dtypes.float8_e4m3)


def kernel(
    time_embed,
    x,
    lin1_w,
    lin1_b,
    lin2_w,
    lin2_b,
    fW_attn_w,
    fW_attn_b,
    fb_attn,
    fW_proj_w,
    fW_proj_b,
    fb_proj,
    _trace=False,
    _times=None,
):
    f64 = np.float64
    # ---- host: time-embedding MLP ----
    t1 = _silu(time_embed.astype(f64) @ lin1_w.astype(f64) + lin1_b.astype(f64))
    t = t1 @ lin2_w.astype(f64) + lin2_b.astype(f64)   # [128]
    t16 = t.astype(np.float16)

    # ---- launch 1: W generation (fp8 LDW-path) ----
    nc_gen = _get("gen", build_gen)
    t32 = t.astype(np.float32)
    s_t = _pow2_scale(np.abs(t32).max())
    t_hi8 = _to_fp8(t32, s_t)
    t_resid = t32 * s_t - t_hi8.astype(np.float32)
    t_lo8 = _to_fp8(t_resid, 16.0)  # extra 4 mantissa bits
    tv_in = np.ascontiguousarray(np.stack(
        [t_hi8, t_lo8], axis=1))  # [128, 2] fp8

    fwa_flat = fW_attn_w.reshape(TEMBED, E * J3).astype(np.float32)
    fwp_flat = fW_proj_w.reshape(TEMBED, E * E).astype(np.float32)
    s_wa = _pow2_scale(np.abs(fwa_flat).max())
    s_wp = _pow2_scale(np.abs(fwp_flat).max())
    fw8 = np.concatenate(
        [_to_fp8(fwa_flat, s_wa), _to_fp8(fwp_flat, s_wp)], axis=1
    )  # [128, 1048576] fp8; cores 0-5 pure attn, 6-7 pure proj
    in_maps = []
    for c in range(NCORES):
        in_maps.append(
            {
                "tv": tv_in,
                "fw": fw8[:, GTOT * c : GTOT * (c + 1)],
            }
        )
    res1 = _run_with_retry(nc_gen, in_maps, trace=_trace)
    if _times is not None:
        _times.append(res1.exec_time_ns)

    slabs = []
    for c in range(NCORES):
        gv = res1.results[c]["g"]  # [128, 2048] f32, cols (hi, lo) pairs
        comb = gv[:, 0::2] + gv[:, 1::2] * (1.0 / 16.0)  # [128, 1024]
        s_w = s_wa if c < 6 else s_wp
        slabs.append(comb.T.reshape(-1) / (s_t * s_w))
    flat = np.concatenate(slabs)  # [1048576]
    Wa = flat[: E * J3].reshape(E, J3)
    Wp = flat[E * J3 :].reshape(E, E)
    Wa = Wa + fW_attn_b.reshape(E, J3)
    Wp = Wp + fW_proj_b.reshape(E, E)
    Wa[:, :512] *= 0.125  # fold 1/sqrt(D) into q columns

    # ---- host: biases ----
    b_attn = (t @ fb_attn.astype(f64).reshape(TEMBED, J3)).astype(np.float32)
    bqk_host = b_attn[:1024].copy()
    bqk_host[:512] *= 0.125
    bqk_in = np.ascontiguousarray(bqk_host.reshape(8, 128).T)
    b_v = b_attn[1024:]
    b_proj = (t @ fb_proj.astype(f64)).astype(np.float32)
    brow = (b_v.astype(f64) @ Wp.astype(f64) + b_proj).astype(np.float16)
    brow_in = np.ascontiguousarray(brow[None, :])
    mask_in = np.triu(np.ones((128, 128), dtype=np.float16))
    ones_in = np.ones((128, 128), dtype=np.float16)
    Wa16 = Wa.astype(np.float16)
    Wp16 = Wp.astype(np.float16)

    # ---- launch 2: attention ----
    if os.environ.get("ATTN_V1", "0") == "1":
        nc_attn = _get("attn", build_attn)
    else:
        nc_attn = _get("attn2", build_attn2)
    in_maps = []
    for c in range(NCORES):
        xt_c = np.ascontiguousarray(
            x[BPC * c : BPC * (c + 1)].reshape(S2, E).T
        )
        in_maps.append(
            {
                "xt": xt_c.astype(np.float16),
                "wa": Wa16,
                "wp": Wp16,
                "bqk": bqk_in,
                "brow": brow_in,
                "mask": mask_in,
                "ones": ones_in,
            }
        )
    res2 = _run_with_retry(nc_attn, in_maps, trace=_trace)
    if _times is not None:
        _times.append(res2.exec_time_ns)

    out = np.empty((B, S, E), dtype=np.float32)
    for c in range(NCORES):
        out[BPC * c : BPC * (c + 1)] = res2.results[c]["out"].reshape(BPC, S, E)
    return out



# revision 29
# speedup vs baseline: 1.1715x; 1.0338x over previous
"""Hypernetwork causal attention (nn_Attention_87926570484382) on 8 TRN2 cores.

Strategy (two launches, batch-sharded attention, gen-sharded hypernet):
  host   : time-embedding MLP -> t [128]; bias assembly; q-scale folding.
  launch1: each core streams 1/8 of fW_attn_w / fW_proj_w through the PE
           (stationary = t replicated across 128 cols, f32r) producing its
           slice of W_attn / W_proj.
  host   : gather W slices, add biases, fold 1/sqrt(D) into the q columns.
  launch2: each core runs full attention for 2 of the 16 batches:
           qk^T = W_attn^T-tiles @ x^T   (heads on partitions)
           scores^T[k,q] per (b,h), exp (no max-sub; causal mask as
           multiply on diagonal tiles), attn@V via ones-extended V (softmax
           denominators land in psum rows 64..127), per-head normalize,
           proj with bias via K=1 matmul, out[s,e].

All matmuls run in float32r (TF32-like, 1 cycle/row); rel err ~2e-4.
"""

import os
import sys

import numpy as np

# ---------------------------------------------------------------------------
# Environment shims (must precede concourse imports in fresh environments)
# ---------------------------------------------------------------------------


def _ensure_axon_hooks():
    """Provide antenv.axon_hooks if the installed antenv lacks it (needed
    only when tracing; harmless otherwise)."""
    try:
        import antenv.axon_hooks  # noqa: F401
        return
    except ImportError:
        pass
    try:
        import antenv
    except ImportError:
        return
    import contextlib
    import ctypes
    import types

    mod = types.ModuleType("antenv.axon_hooks")
    mod._HOOK = None
    mod._TRIED = False

    def set_axon_ntff_profile_hook(hook):
        mod._HOOK = hook

    def _build(so_path):
        lib = ctypes.CDLL(so_path)
        if not hasattr(lib, "axon_start_nrt_profile"):
            return None
        lib.axon_start_nrt_profile.argtypes = [
            ctypes.POINTER(ctypes.c_int64),
            ctypes.c_size_t,
        ]
        lib.axon_start_nrt_profile.restype = ctypes.c_int64
        lib.axon_stop_nrt_profile.argtypes = [ctypes.c_char_p]
        lib.axon_stop_nrt_profile.restype = ctypes.c_int64

        @contextlib.contextmanager
        def _hook(output_dir, device_ids):
            import jax

            jax.devices()
            if device_ids:
                ids = (ctypes.c_int64 * len(device_ids))(*device_ids)
                rc = lib.axon_start_nrt_profile(ids, len(device_ids))
            else:
                rc = lib.axon_start_nrt_profile(None, 0)
            if rc != 0:
                raise RuntimeError(f"axon_start_nrt_profile rc={rc}")
            try:
                yield
            finally:
                n = lib.axon_stop_nrt_profile(str(output_dir).encode())
                print(f"profile: {n} file(s) -> {output_dir}", file=sys.stderr)

        return _hook

    def get_axon_ntff_profile_hook():
        if mod._HOOK is None and not mod._TRIED:
            mod._TRIED = True
            p = "/opt/axon/libaxon_pjrt.so"
            if os.path.exists(p):
                try:
                    mod._HOOK = _build(p)
                except OSError:
                    mod._HOOK = None
        return mod._HOOK

    mod.set_axon_ntff_profile_hook = set_axon_ntff_profile_hook
    mod.get_axon_ntff_profile_hook = get_axon_ntff_profile_hook
    sys.modules["antenv.axon_hooks"] = mod
    antenv.axon_hooks = mod


_ensure_axon_hooks()

import concourse.bass as bass  # noqa: E402
import concourse.mybir as mybir  # noqa: E402
from concourse import tile as _tile  # noqa: E402
from concourse.tile import TileContext, add_dep_helper  # noqa: E402
from concourse.vector_clock import ScopedClock  # noqa: E402
from concourse.bass_utils import run_bass_kernel_spmd  # noqa: E402

F32 = mybir.dt.float32
F32R = mybir.dt.float32r
F16 = mybir.dt.float16
FP8 = mybir.dt.float8e4

# problem constants (hardcoded per harness contract)
SIN_DIM, TEMBED = 64, 128
E, H, D = 512, 8, 64
B, S = 16, 1024
NCORES = 8
SKIP_NR = os.environ.get("SKIP_NR", "1") == "1"
BPC = B // NCORES          # batches per core
S2 = BPC * S               # 2048 rows per core
J3 = 3 * H * D             # 1536
EJ_A = E * J3 // NCORES    # 98304  fW_attn slab cols per core
EJ_P = E * E // NCORES     # 32768  fW_proj slab cols per core
GTOT = (E * J3 + E * E) // NCORES   # 131072 slab cols per core (concat split)
GT_TILES = GTOT // 128              # 1024 stationary tiles per core
NT = 2                              # t columns per matmul (hi, lo fp8)
GROUP_TILES = 512 // NT             # tiles per psum bank

# ---------------------------------------------------------------------------
# Tile framework workarounds: this walrus accepts at most ONE semaphore wait
# and one update per instruction.
# ---------------------------------------------------------------------------

_NOP_CTR = [0]


def _patched_drain_and_barrier(self, tick_clock, wait_clock):
    carrier = self.nc.sync.nop(nofuse=True)
    wait_clock.add_sem_waits(
        carrier.ins, ScopedClock({None: tick_clock.global_clock})
    )
    si = carrier.ins.sync_info
    waits = list(si.on_wait) if si and si.on_wait else []
    if len(waits) > 1:
        carrier.ins.sync_info = mybir.SyncInfo(
            on_wait=waits[:1],
            on_update=list(si.on_update) if si and si.on_update else [],
        )
        for w in waits[1:]:
            extra = self.nc.sync.nop(nofuse=True)
            extra.ins.sync_info = mybir.SyncInfo(on_wait=[w], on_update=[])
    self.nc.sync.drain()
    self.nc.all_engine_barrier()
    assert self.sems is not None
    popped = self.nc._tile_sem_poison_stack.pop()
    assert popped is self._sem_poison
    self.nc.clear_and_free_semaphores(list(self.sems.allocated().values()))
    self.nc.all_engine_barrier()


_tile.TileContext._drain_and_barrier = _patched_drain_and_barrier


def _split_multi_waits(nc):
    for f in nc.m.functions:
        for blk in f.blocks:
            out = []
            changed = False
            for inst in blk.instructions:
                si = inst.sync_info
                waits = list(si.on_wait) if si and si.on_wait else []
                updates = list(si.on_update) if si and si.on_update else []
                is_dma = "DMA" in type(inst).__name__
                if len(waits) > 1:
                    changed = True
                    for w in waits[:-1]:
                        _NOP_CTR[0] += 1
                        nop = mybir.InstNoOp(
                            name=f"wsplit_{_NOP_CTR[0]}", ins=[], outs=[]
                        )
                        nop.engine = inst.engine
                        nop.sync_info = mybir.SyncInfo(on_wait=[w], on_update=[])
                        out.append(nop)
                    waits = [waits[-1]]
                    inst.sync_info = mybir.SyncInfo(
                        on_wait=waits, on_update=updates
                    )
                out.append(inst)
                if len(updates) > 1:
                    if is_dma:
                        raise AssertionError(
                            f"DMA {inst.name} has {len(updates)} updates"
                        )
                    changed = True
                    inst.sync_info = mybir.SyncInfo(
                        on_wait=waits, on_update=[updates[0]]
                    )
                    for u in updates[1:]:
                        _NOP_CTR[0] += 1
                        nop = mybir.InstNoOp(
                            name=f"usplit_{_NOP_CTR[0]}", ins=[], outs=[]
                        )
                        nop.engine = inst.engine
                        nop.sync_info = mybir.SyncInfo(on_wait=[], on_update=[u])
                        out.append(nop)
            if changed:
                blk.instructions = out
    return nc


# ---------------------------------------------------------------------------
# Launch 1: hypernetwork weight generation
# ---------------------------------------------------------------------------


def build_gen():
    """Per core: W_slab[c] = sum_k t[k] * fW_slab[k, c] for a 1/8 slice of
    the concatenated (fW_attn_w | fW_proj_w) column space.

    Flipped matmul orientation: the fp8 fW tile [128k, 128j] is the
    STATIONARY operand (FWL-accelerated LDWEIGHTS carries the bandwidth)
    and t is the moving operand ([128, 2]: hi/lo fp8 split), so each
    matmul emits a compact [128, 2] psum column pair -- no replicated
    output, copies shrink from 64 MB to 1 MB per core.  Host combines
    hi + lo/16 and rescales."""
    nc = bass.Bass()
    fw = nc.dram_tensor("fw", [TEMBED, GTOT], FP8, kind="ExternalInput")
    tv = nc.dram_tensor("tv", [TEMBED, NT], FP8, kind="ExternalInput")
    g = nc.dram_tensor("g", [128, GT_TILES * NT], F32, kind="ExternalOutput")

    GCHUNK = 8192           # fW cols per DMA chunk (1 MB fp8)
    TPC = GCHUNK // 128     # 64 stationary tiles per chunk

    with TileContext(nc) as tc:
        with (
            tc.tile_pool(name="cst", bufs=1) as cst,
            tc.tile_pool(name="sb", bufs=5) as sb,
            tc.tile_pool(name="gs", bufs=2) as gsp,
            tc.tile_pool(name="ps", bufs=2, space="PSUM") as ps,
        ):
            tt = cst.tile([TEMBED, NT], FP8)
            nc.sync.dma_start(out=tt, in_=tv[:, :])

            pg = None
            for chunk in range(GTOT // GCHUNK):
                ft = sb.tile([TEMBED, GCHUNK], FP8, tag="fw", name=f"fw{chunk}")
                # alternate the two HWDGE rings so fixed DMA latencies overlap
                eng = nc.sync if chunk % 2 == 0 else nc.scalar
                eng.dma_start(
                    out=ft, in_=fw[:, chunk * GCHUNK : (chunk + 1) * GCHUNK]
                )
                for i in range(TPC):
                    c = chunk * TPC + i
                    grp, off = divmod(c, GROUP_TILES)
                    if off == 0:
                        pg = ps.tile([128, 512], F32, tag="pg", name=f"pg{grp}")
                    nc.tensor.matmul(
                        pg[:, NT * off : NT * (off + 1)],
                        ft[:, 128 * i : 128 * (i + 1)],
                        tt,
                        start=True,
                        stop=True,
                    )
                    if off == GROUP_TILES - 1:
                        gt = gsp.tile([128, 512], F32, tag="gs", name=f"gs{grp}")
                        nc.vector.tensor_copy(gt, pg)
                        # outputs ride SWDGE so they never head-of-line
                        # block the input chunk stream
                        nc.gpsimd.dma_start(
                            out=g[:, 512 * grp : 512 * (grp + 1)], in_=gt
                        )
    _split_multi_waits(nc)
    return nc


# ---------------------------------------------------------------------------
# Launch 2: attention for 2 batches per core
# ---------------------------------------------------------------------------


def _act_recip_seed(nc, out, in_):
    eng = nc.scalar
    imm = lambda v: mybir.ImmediateValue(dtype=mybir.dt.float32, value=v)
    return eng.add_instruction(
        mybir.InstActivation(
            name=eng.bass.get_next_instruction_name(),
            func=mybir.ActivationFunctionType.Reciprocal,
            ins=[eng.lower_ap(in_), imm(0.0), imm(1.0), imm(0.0)],
            outs=[eng.lower_ap(out)],
        )
    )


def build_attn():
    nc = bass.Bass()
    xt = nc.dram_tensor("xt", [E, S2], F16, kind="ExternalInput")
    wa = nc.dram_tensor("wa", [E, J3], F16, kind="ExternalInput")
    wp = nc.dram_tensor("wp", [E, E], F16, kind="ExternalInput")
    bqk = nc.dram_tensor("bqk", [128, 8], F32, kind="ExternalInput")
    brow = nc.dram_tensor("brow", [1, E], F16, kind="ExternalInput")
    mask = nc.dram_tensor("mask", [128, 128], F16, kind="ExternalInput")
    ones = nc.dram_tensor("ones", [128, 128], F16, kind="ExternalInput")
    out = nc.dram_tensor("out", [S2, E], F32, kind="ExternalOutput")

    NQT = S // 128           # 8 q/k tiles per batch
    NET = E // 128           # 4 e tiles

    with TileContext(nc) as tc:
        with (
            tc.tile_pool(name="cst", bufs=1) as cst,
            tc.tile_pool(name="qk", bufs=1) as qkp,
            tc.tile_pool(name="vx", bufs=1) as vxp,
            tc.tile_pool(name="ot", bufs=1) as otp,
            tc.tile_pool(name="ou", bufs=1) as oup,
            tc.tile_pool(name="wk", bufs=3) as wk,
            tc.tile_pool(name="ex", bufs=4) as exp_pool,
            tc.tile_pool(name="ps", bufs=3, space="PSUM") as ps,
            tc.tile_pool(name="po", bufs=5, space="PSUM") as pop,
        ):
            # resident inputs
            xts = []
            was = []
            wps = []
            for et in range(NET):
                t_ = cst.tile([128, S2], F16, tag=f"xt{et}")
                nc.sync.dma_start(out=t_, in_=xt[128 * et : 128 * (et + 1), :])
                xts.append(t_)
                t_ = cst.tile([128, J3], F16, tag=f"wa{et}")
                nc.sync.dma_start(out=t_, in_=wa[128 * et : 128 * (et + 1), :])
                was.append(t_)
                t_ = cst.tile([128, E], F16, tag=f"wp{et}")
                nc.sync.dma_start(out=t_, in_=wp[128 * et : 128 * (et + 1), :])
                wps.append(t_)
            bqk_t = cst.tile([128, 8], F32)
            nc.sync.dma_start(out=bqk_t, in_=bqk[:, :])
            brow_t = cst.tile([1, E], F16)
            nc.sync.dma_start(out=brow_t, in_=brow[:, :])
            mask_t = cst.tile([128, 128], F16)
            nc.sync.dma_start(out=mask_t, in_=mask[:, :])
            ones1 = cst.tile([1, 128], F16)
            nc.sync.dma_start(out=ones1, in_=ones[0:1, :])
            # persistent v_ext tiles: [128 k-rows, 8h * (64 v | 64 ones)];
            # ones columns are constant, v columns rewritten per batch
            vxs = []
            for st in range(S // 128):
                v_ = vxp.tile([128, 8 * 128], F16, tag=f"vx{st}", name=f"vx{st}")
                for h in range(H):
                    nc.sync.dma_start(
                        out=v_[:, 128 * h + 64 : 128 * (h + 1)],
                        in_=ones[:, 0:64],
                    )
                vxs.append(v_)

            state = {}

            def qkv_phase(b):
                s0 = b * S
                qkts = []
                for m in range(8):
                    qkts.append(qkp.tile([128, S], F16, tag=f"qkT{m}", name=f"qkT{m}_{b}"))
                for m in range(8):
                    for sc in range(S // 512):
                        pq = ps.tile([128, 512], F32, tag="ps", name=f"pq_{b}_{m}_{sc}")
                        for et in range(NET):
                            nc.tensor.matmul(
                                pq,
                                was[et][:, 128 * m : 128 * (m + 1)],
                                xts[et][:, s0 + 512 * sc : s0 + 512 * (sc + 1)],
                                start=(et == 0),
                                stop=(et == NET - 1),
                            )
                        nc.vector.tensor_scalar_add(
                            qkts[m][:, 512 * sc : 512 * (sc + 1)],
                            pq,
                            bqk_t[:, m : m + 1],
                        )
                for st in range(NQT):
                    pv = ps.tile([128, 512], F32, tag="ps", name=f"pv_{b}_{st}")
                    for et in range(NET):
                        nc.tensor.matmul(
                            pv,
                            xts[et][:, s0 + 128 * st : s0 + 128 * (st + 1)],
                            was[et][:, 1024:1536],
                            start=(et == 0),
                            stop=(et == NET - 1),
                        )
                    for h in range(H):
                        if h % 2 == 0:
                            nc.vector.tensor_copy(
                                vxs[st][:, 128 * h : 128 * h + 64],
                                pv[:, 64 * h : 64 * (h + 1)],
                            )
                        else:
                            nc.scalar.copy(
                                vxs[st][:, 128 * h : 128 * h + 64],
                                pv[:, 64 * h : 64 * (h + 1)],
                            )
                state[b] = {"qkts": qkts}

            def attn_phase(b):
                st_ = state[b]
                qkts = st_["qkts"]
                ots = [
                    otp.tile([128, S], F16, tag=f"oT{ht}", name=f"oT{ht}_{b}")
                    for ht in range(NET)
                ]
                osm = [
                    oup.tile([128, S], F32, tag=f"ou{h}", name=f"ou{h}_{b}")
                    for h in range(H)
                ]
                st_.update(ots=ots, osm=osm, exps=[])
                # sandwich: this batch's exps must follow the previous
                # batch's reciprocal seeds (ACT table thrash avoidance)
                prev = state.get(b - 1)
                if prev and prev.get("seeds"):
                    st_["first_exp_dep"] = prev["seeds"][-1]
                for h in range(H):
                    kt_tile = qkts[4 + h // 2]
                    qt_tile = qkts[h // 2]
                    prow = 64 * (h % 2)
                    po0 = pop.tile([128, 512], F32, tag="po", name=f"po0_{b}_{h}")
                    po1 = pop.tile([128, 512], F32, tag="po", name=f"po1_{b}_{h}")
                    pos = (po0, po1)

                    def scores_exp(j):
                        expt = exp_pool.tile(
                            [128, S], F16, tag="expt", name=f"expt_{b}_{h}_{j}"
                        )
                        for qc in range(2):
                            c0 = max(512 * qc, 128 * j)
                            c1 = 512 * (qc + 1)
                            if c0 >= c1:
                                continue
                            pst = ps.tile(
                                [128, 512], F32, tag="ps", name=f"pst_{b}_{h}_{j}_{qc}"
                            )
                            nc.tensor.matmul(
                                pst[:, 0 : c1 - c0],
                                kt_tile[prow : prow + 64, 128 * j : 128 * (j + 1)],
                                qt_tile[prow : prow + 64, c0:c1],
                                start=True,
                                stop=True,
                            )
                            ei = nc.scalar.activation(
                                expt[:, c0:c1],
                                pst[:, 0 : c1 - c0],
                                func=mybir.ActivationFunctionType.Exp,
                            )
                            st_["exps"].append(ei.ins)
                        nc.vector.tensor_mul(
                            expt[:, 128 * j : 128 * (j + 1)],
                            expt[:, 128 * j : 128 * (j + 1)],
                            mask_t,
                        )
                        return expt

                    expts = {0: scores_exp(0), 1: scores_exp(1)}
                    for j in range(NQT):
                        if j + 2 < NQT:
                            expts[j + 2] = scores_exp(j + 2)
                        expt = expts.pop(j)
                        for qc in range(2):
                            c0 = max(512 * qc, 128 * j)
                            c1 = 512 * (qc + 1)
                            if c0 >= c1:
                                continue
                            nc.tensor.matmul(
                                pos[qc][:, c0 - 512 * qc : c1 - 512 * qc],
                                vxs[j][:, 128 * h : 128 * (h + 1)],
                                expt[:, c0:c1],
                                start=(j == 0),
                                stop=(j == NQT - 1 if qc == 1 else j == 3),
                            )
                    for qc in range(2):
                        nc.vector.tensor_copy(
                            osm[h][:, 512 * qc : 512 * (qc + 1)],
                            pos[qc][:, :],
                        )

            def norm_phase(b):
                st_ = state[b]
                ots, osm = st_["ots"], st_["osm"]
                y0s = {}
                st_["seeds"] = []
                for h in range(H):
                    y0s[h] = wk.tile([64, S], F32, tag="y0", name=f"y0_{b}_{h}")
                    si = _act_recip_seed(nc, y0s[h], osm[h][64:128, :])
                    st_["seeds"].append(si.ins)
                for h in range(H):
                    prow = 64 * (h % 2)
                    if SKIP_NR:
                        rec = y0s[h]
                    else:
                        tvar = wk.tile([64, S], F32, tag="tvar", name=f"tv_{b}_{h}")
                        nc.vector.tensor_mul(tvar, osm[h][64:128, :], y0s[h])
                        nc.vector.tensor_scalar(
                            out=tvar, in0=tvar, scalar1=-1.0, scalar2=2.0,
                            op0=mybir.AluOpType.mult, op1=mybir.AluOpType.add,
                        )
                        rec = wk.tile([64, S], F32, tag="rec", name=f"rec_{b}_{h}")
                        nc.vector.tensor_mul(rec, tvar, y0s[h])
                    nc.vector.tensor_mul(
                        ots[h // 2][prow : prow + 64, :], osm[h][0:64, :], rec
                    )

            def proj_phase(b, st_lo=0, st_hi=NQT):
                s0 = b * S
                ots = state[b]["ots"]
                for st in range(st_lo, st_hi):
                    pp = ps.tile([128, 512], F32, tag="ps", name=f"pp_{b}_{st}")
                    for ht in range(NET):
                        nc.tensor.matmul(
                            pp,
                            ots[ht][:, 128 * st : 128 * (st + 1)],
                            wps[ht],
                            start=(ht == 0),
                            stop=False,
                        )
                    nc.tensor.matmul(
                        pp, ones1, brow_t, start=False, stop=True
                    )
                    ob = wk.tile([128, E], F32, tag="ob", name=f"ob_{b}_{st}")
                    nc.vector.tensor_copy(ob, pp)
                    nc.sync.dma_start(
                        out=out[s0 + 128 * st : s0 + 128 * (st + 1), :], in_=ob
                    )

            # batch-level software pipeline: norm(b)/proj(b) overlap the
            # next batch's dense PE phases
            qkv_phase(0)
            attn_phase(0)
            qkv_phase(1)
            norm_phase(0)
            attn_phase(1)
            proj_phase(0, 0, 4)
            norm_phase(1)
            proj_phase(0, 4, NQT)
            proj_phase(1)
    _split_multi_waits(nc)
    return nc


# ---------------------------------------------------------------------------
# Launch 2 (v2): attention, restructured for HAM warmth + merged exp calls
# ---------------------------------------------------------------------------


def build_attn2():
    """Per core: full causal attention for 2 batches.

    vs v1: (1) head-pair score matmuls issued adjacently so the K=64 row
    tiles (partitions 0-63 / 64-127) run concurrently on the PE;
    (2) scores for a (pair, j) land in one 4-bank psum tile and are
    exponentiated by ONE ACT call via a [128, 2, N] access pattern
    (6 calls/pair instead of 28); (3) softmax normalize via DVE
    reciprocal_approx_fast straight out of psum (no Reciprocal table
    thrash, no replicated copies); (4) qkv/proj chunks of the
    neighbouring batch are interleaved into the pair loop to fill the
    PE gaps left by the exp dependency chain."""
    nc = bass.Bass()
    xt = nc.dram_tensor("xt", [E, S2], F16, kind="ExternalInput")
    wa = nc.dram_tensor("wa", [E, J3], F16, kind="ExternalInput")
    wp = nc.dram_tensor("wp", [E, E], F16, kind="ExternalInput")
    bqk = nc.dram_tensor("bqk", [128, 8], F32, kind="ExternalInput")
    brow = nc.dram_tensor("brow", [1, E], F16, kind="ExternalInput")
    mask = nc.dram_tensor("mask", [128, 128], F16, kind="ExternalInput")
    ones = nc.dram_tensor("ones", [128, 128], F16, kind="ExternalInput")
    out = nc.dram_tensor("out", [S2, E], F32, kind="ExternalOutput")

    NQT = S // 128           # 8 k-tiles (and q-tiles) per batch
    NET = E // 128           # 4 e-tiles

    with TileContext(nc) as tc:
        with (
            tc.tile_pool(name="cst", bufs=1) as cst,
            tc.tile_pool(name="qk", bufs=1) as qkp,
            tc.tile_pool(name="vx", bufs=1) as vxp,
            tc.tile_pool(name="ot", bufs=1) as otp,
            tc.tile_pool(name="ex", bufs=2) as exp_pool,
            tc.tile_pool(name="os", bufs=1) as osp,
            tc.tile_pool(name="wk", bufs=2) as wk,
            tc.tile_pool(name="sp", bufs=1, space="PSUM") as spp,
            tc.tile_pool(name="po", bufs=2, space="PSUM") as pop,
            tc.tile_pool(name="ps", bufs=2, space="PSUM") as ps,
        ):
            # ---- streamed input residency (few big DMAs; wa first so the
            # first qk matmuls can start as soon as x batch 0 lands) ----
            xts = [
                cst.tile([128, S2], F16, tag=f"xt{et}", name=f"xts{et}")
                for et in range(NET)
            ]
            was = [
                cst.tile([128, J3], F16, tag=f"wa{et}", name=f"was{et}")
                for et in range(NET)
            ]
            wps = [
                cst.tile([128, E], F16, tag=f"wp{et}", name=f"wps{et}")
                for et in range(NET)
            ]
            bqk_t = cst.tile([128, 8], F32)
            nc.sync.dma_start(out=bqk_t, in_=bqk[:, :])
            for et in range(NET):
                nc.scalar.dma_start(
                    out=was[et], in_=wa[128 * et : 128 * (et + 1), :]
                )
            for et in range(NET):
                nc.sync.dma_start(
                    out=xts[et][:, 0:S], in_=xt[128 * et : 128 * (et + 1), 0:S]
                )
            for et in range(NET):
                nc.sync.dma_start(
                    out=xts[et][:, S:S2], in_=xt[128 * et : 128 * (et + 1), S:S2]
                )
            mask_t = cst.tile([128, 128], F16)
            nc.sync.dma_start(out=mask_t, in_=mask[:, :])
            ones1 = cst.tile([1, 128], F16)
            nc.sync.dma_start(out=ones1, in_=ones[0:1, :])
            brow_t = cst.tile([1, E], F16)
            nc.sync.dma_start(out=brow_t, in_=brow[:, :])
            for et in range(NET):
                nc.scalar.dma_start(
                    out=wps[et], in_=wp[128 * et : 128 * (et + 1), :]
                )
            # v_ext tiles: [128 k, 8h * (64 v | 64 ones)]; per batch so the
            # next batch's v generation never WAR-blocks on this batch's
            # attn@V reads.  ones cols come from a gpsimd memset, v cols
            # are overwritten by the v-generation copy.
            vxs = {}
            for b in range(BPC):
                vxs[b] = []
                for st in range(NQT):
                    v_ = vxp.tile(
                        [128, 8 * 128], F16, tag=f"vx{st}_{b}", name=f"vx{st}_{b}"
                    )
                    nc.gpsimd.memset(v_, 1.0)
                    vxs[b].append(v_)

            state = {}

            # ---- phase pieces, emitted in interleaved order below ----

            def qkv_qk_chunk(b, m, sc):
                """qk^T[128 cols of m-chunk, 512 s] for batch b."""
                s0 = b * S
                qkts = state[b]["qkts"]
                pq = ps.tile([128, 512], F32, tag="ps", name=f"pq_{b}_{m}_{sc}")
                for et in range(NET):
                    nc.tensor.matmul(
                        pq,
                        was[et][:, 128 * m : 128 * (m + 1)],
                        xts[et][:, s0 + 512 * sc : s0 + 512 * (sc + 1)],
                        start=(et == 0),
                        stop=(et == NET - 1),
                    )
                nc.vector.tensor_scalar_add(
                    qkts[m][:, 512 * sc : 512 * (sc + 1)],
                    pq,
                    bqk_t[:, m : m + 1],
                )

            def qkv_v_chunk(b, st):
                """v rows for s-tile st of batch b, scattered into vxs."""
                s0 = b * S
                pv = ps.tile([128, 512], F32, tag="ps", name=f"pv_{b}_{st}")
                for et in range(NET):
                    nc.tensor.matmul(
                        pv,
                        xts[et][:, s0 + 128 * st : s0 + 128 * (st + 1)],
                        was[et][:, 1024:1536],
                        start=(et == 0),
                        stop=(et == NET - 1),
                    )
                # one strided copy: [128, 8, 64] psum -> v cols of vxs[b][st]
                nc.vector.tensor_copy(
                    vxs[b][st][:, 0 : 8 * 128].rearrange(
                        "p (h t) -> p h t", h=8
                    )[:, :, 0:64],
                    pv.rearrange("p (h d) -> p h d", h=8),
                )

            def scores_exp(b, p, j):
                """Scores + exp for head pair p, k-tile j: two concurrent
                K=64 matmul chains into one 4-bank psum tile, one exp."""
                qkts = state[b]["qkts"]
                kt, qt = qkts[4 + p], qkts[p]
                c0 = 128 * j
                n = S - c0
                sp = spp.tile([128, 2048], F32, tag="spair", name=f"sp_{b}_{p}_{j}")
                for lo, hi in ((c0, 512), (max(512, c0), 1024)):
                    if lo >= hi:
                        continue
                    for hh in range(2):
                        nc.tensor.matmul(
                            sp[:, 1024 * hh + lo : 1024 * hh + hi],
                            kt[64 * hh : 64 * hh + 64, c0 : c0 + 128],
                            qt[64 * hh : 64 * hh + 64, lo:hi],
                            start=True,
                            stop=True,
                        )
                ex = exp_pool.tile(
                    [128, 2 * n], F16, tag=f"ex{j}", name=f"ex_{b}_{p}_{j}"
                )
                ei = nc.scalar.activation(
                    ex.rearrange("p (h q) -> p h q", h=2),
                    sp.rearrange("p (h q) -> p h q", h=2)[:, :, c0:S],
                    func=mybir.ActivationFunctionType.Exp,
                )
                state[b].setdefault("exps", []).append(ei.ins)
                # causal mask on the diagonal 128-col block of both heads
                # (gpsimd: SBUF-only op on an otherwise idle engine)
                nc.gpsimd.tensor_mul(
                    ex.rearrange("p (h q) -> p h q", h=2)[:, :, 0:128],
                    ex.rearrange("p (h q) -> p h q", h=2)[:, :, 0:128],
                    mask_t.unsqueeze(1).broadcast_to((128, 2, 128)),
                )
                state[b][f"ex_{p}_{j}"] = ex

            def attnv(b, p):
                """attn @ V_ext for pair p (2 q-passes); unnormalized o and
                replicated denominators staged to SBUF (f16) so psum frees
                immediately and the reciprocals can batch at batch end."""
                for qc in range(2):
                    for hh in range(2):
                        po = pop.tile(
                            [128, 512], F32, tag="po", name=f"po_{b}_{p}_{qc}_{hh}"
                        )
                        jmax = 4 if qc == 0 else NQT
                        for j in range(jmax):
                            ex = state[b][f"ex_{p}_{j}"]
                            n = S - 128 * j
                            lo = max(512 * qc, 128 * j) - 128 * j
                            hi = 512 * (qc + 1) - 128 * j
                            nc.tensor.matmul(
                                po[:, lo + 128 * j - 512 * qc : 512],
                                vxs[b][j][
                                    :, 128 * (2 * p + hh) : 128 * (2 * p + hh + 1)
                                ],
                                ex[:, n * hh + lo : n * hh + hi],
                                start=(j == 0),
                                stop=(j == jmax - 1),
                            )
                        nc.vector.tensor_copy(
                            state[b]["osm"][2 * p + hh][:, 512 * qc : 512 * (qc + 1)],
                            po,
                        )

            def norm(b):
                """Batched softmax normalization: one ACT Reciprocal window
                per batch (single table-set switch), then DVE muls."""
                ots, osm = state[b]["ots"], state[b]["osm"]
                rcps = {}
                for h in range(H):
                    rcp = wk.tile([64, S], F16, tag="rcp", name=f"rcp_{b}_{h}")
                    ri = _act_recip_seed(nc, rcp, osm[h][64:128, :])
                    state[b].setdefault("recips", []).append(ri.ins)
                    rcps[h] = rcp
                for h in range(H):
                    nc.vector.tensor_mul(
                        ots[h // 2][64 * (h % 2) : 64 * (h % 2) + 64, :],
                        osm[h][0:64, :],
                        rcps[h],
                    )

            def proj_chunk(b, st):
                s0 = b * S
                ots = state[b]["ots"]
                pp = ps.tile([128, 512], F32, tag="ps", name=f"pp_{b}_{st}")
                for pt in range(NET):
                    nc.tensor.matmul(
                        pp,
                        ots[pt][:, 128 * st : 128 * (st + 1)],
                        wps[pt],
                        start=(pt == 0),
                        stop=False,
                    )
                nc.tensor.matmul(pp, ones1, brow_t, start=False, stop=True)
                ob = wk.tile([128, E], F32, tag="ob", name=f"ob_{b}_{st}")
                nc.vector.tensor_copy(ob, pp)
                nc.sync.dma_start(
                    out=out[s0 + 128 * st : s0 + 128 * (st + 1), :], in_=ob
                )

            def init_batch(b):
                state[b] = {
                    "qkts": [
                        qkp.tile(
                            [128, S], F16, tag=f"qkT{m}_{b}", name=f"qkT{m}_{b}"
                        )
                        for m in range(8)
                    ],
                    "ots": [
                        otp.tile([128, S], F16, tag=f"oT{p}", name=f"oT{p}_{b}")
                        for p in range(4)
                    ],
                    "osm": [
                        osp.tile(
                            [128, S], F16, tag=f"osm{h}_{b}", name=f"osm{h}_{b}"
                        )
                        for h in range(H)
                    ],
                }

            # ---- emission schedule ----
            # Both batches' qkv up front (dense PE work while inputs
            # stream), then the 8 head-pairs of the two batches
            # interleaved so the ACT-bound exp stream always has PE work
            # (other pair's scores / attn@V) to hide under.  All
            # reciprocals pinned after the last exp: exactly two ACT
            # table-set switches for the whole launch.
            init_batch(0)
            init_batch(1)
            for b in range(BPC):
                for m in range(8):
                    for sc in range(2):
                        qkv_qk_chunk(b, m, sc)
                for st in range(NQT):
                    qkv_v_chunk(b, st)
            glist = [(b, p) for p in range(4) for b in range(BPC)]
            for gi, (b, p) in enumerate(glist):
                if gi >= 2:
                    attnv(*glist[gi - 2])
                for j in range(NQT):
                    scores_exp(b, p, j)
            attnv(*glist[-2])
            attnv(*glist[-1])
            norm(0)
            for st in range(NQT):
                proj_chunk(0, st)
            norm(1)
            for st in range(NQT):
                proj_chunk(1, st)
            # pin every reciprocal after the final exp (ACT ordering only)
            last_exp = state[glist[-1][0]]["exps"][-1]
            for b in range(BPC):
                for ri in state[b]["recips"]:
                    add_dep_helper(ri, last_exp, sync=False,
                                   reason="act table-set batching")
    _split_multi_waits(nc)
    return nc


# ---------------------------------------------------------------------------
# Host orchestration
# ---------------------------------------------------------------------------

_CACHE = {}


def _get(name, builder):
    if name not in _CACHE:
        _CACHE[name] = builder()
    return _CACHE[name]


def _run_with_retry(nc, in_maps, trace=False, tries=3):
    import time as _time

    last = None
    for attempt in range(tries):
        try:
            return run_bass_kernel_spmd(
                nc, in_maps, core_ids=list(range(NCORES)), trace=trace
            )
        except Exception as e:  # transient NRT_EXEC_UNIT_UNRECOVERABLE etc.
            last = e
            _time.sleep(2.0 * (attempt + 1))
    raise last


def _silu(v):
    return v / (1.0 + np.exp(-v))


def _pow2_scale(maxv, target=224.0):
    """Largest power of two s with maxv * s <= ~target (fp8e4 max 240)."""
    if maxv <= 0:
        return 1.0
    return float(2.0 ** np.floor(np.log2(target / maxv)))


def _to_fp8(x32, scale):
    import ml_dtypes

    return np.clip(x32 * scale, -240.0, 240.0).astype(ml_dtypes.float8_e4m3)


def kernel(
    time_embed,
    x,
    lin1_w,
    lin1_b,
    lin2_w,
    lin2_b,
    fW_attn_w,
    fW_attn_b,
    fb_attn,
    fW_proj_w,
    fW_proj_b,
    fb_proj,
    _trace=False,
    _times=None,
):
    f64 = np.float64
    # ---- host: time-embedding MLP ----
    t1 = _silu(time_embed.astype(f64) @ lin1_w.astype(f64) + lin1_b.astype(f64))
    t = t1 @ lin2_w.astype(f64) + lin2_b.astype(f64)   # [128]
    t16 = t.astype(np.float16)

    # ---- launch 1: W generation (fp8 LDW-path) ----
    nc_gen = _get("gen", build_gen)
    t32 = t.astype(np.float32)
    s_t = _pow2_scale(np.abs(t32).max())
    t_hi8 = _to_fp8(t32, s_t)
    t_resid = t32 * s_t - t_hi8.astype(np.float32)
    t_lo8 = _to_fp8(t_resid, 16.0)  # extra 4 mantissa bits
    tv_in = np.ascontiguousarray(np.stack(
        [t_hi8, t_lo8], axis=1))  # [128, 2] fp8

    fwa_flat = fW_attn_w.reshape(TEMBED, E * J3).astype(np.float32)
    fwp_flat = fW_proj_w.reshape(TEMBED, E * E).astype(np.float32)
    s_wa = _pow2_scale(np.abs(fwa_flat).max())
    s_wp = _pow2_scale(np.abs(fwp_flat).max())
    fw8 = np.concatenate(
        [_to_fp8(fwa_flat, s_wa), _to_fp8(fwp_flat, s_wp)], axis=1
    )  # [128, 1048576] fp8; cores 0-5 pure attn, 6-7 pure proj
    in_maps = []
    for c in range(NCORES):
        in_maps.append(
            {
                "tv": tv_in,
                "fw": fw8[:, GTOT * c : GTOT * (c + 1)],
            }
        )
    res1 = _run_with_retry(nc_gen, in_maps, trace=_trace)
    if _times is not None:
        _times.append(res1.exec_time_ns)

    slabs = []
    for c in range(NCORES):
        gv = res1.results[c]["g"]  # [128, 2048] f32, cols (hi, lo) pairs
        comb = gv[:, 0::2] + gv[:, 1::2] * (1.0 / 16.0)  # [128, 1024]
        s_w = s_wa if c < 6 else s_wp
        slabs.append(comb.T.reshape(-1) / (s_t * s_w))
    flat = np.concatenate(slabs)  # [1048576]
    Wa = flat[: E * J3].reshape(E, J3)
    Wp = flat[E * J3 :].reshape(E, E)
    Wa = Wa + fW_attn_b.reshape(E, J3)
    Wp = Wp + fW_proj_b.reshape(E, E)
    Wa[:, :512] *= 0.125  # fold 1/sqrt(D) into q columns

    # ---- host: biases ----
    b_attn = (t @ fb_attn.astype(f64).reshape(TEMBED, J3)).astype(np.float32)
    bqk_host = b_attn[:1024].copy()
    bqk_host[:512] *= 0.125
    bqk_in = np.ascontiguousarray(bqk_host.reshape(8, 128).T)
    b_v = b_attn[1024:]
    b_proj = (t @ fb_proj.astype(f64)).astype(np.float32)
    brow = (b_v.astype(f64) @ Wp.astype(f64) + b_proj).astype(np.float16)
    brow_in = np.ascontiguousarray(brow[None, :])
    mask_in = np.triu(np.ones((128, 128), dtype=np.float16))
    ones_in = np.ones((128, 128), dtype=np.float16)
    Wa16 = Wa.astype(np.float16)
    Wp16 = Wp.astype(np.float16)

    # ---- launch 2: attention ----
    if os.environ.get("ATTN_V1", "0") == "1":
        nc_attn = _get("attn", build_attn)
    else:
        nc_attn = _get("attn2", build_attn2)
    in_maps = []
    for c in range(NCORES):
        xt_c = np.ascontiguousarray(
            x[BPC * c : BPC * (c + 1)].reshape(S2, E).T
        )
        in_maps.append(
            {
                "xt": xt_c.astype(np.float16),
                "wa": Wa16,
                "wp": Wp16,
                "bqk": bqk_in,
                "brow": brow_in,
                "mask": mask_in,
                "ones": ones_in,
            }
        )
    res2 = _run_with_retry(nc_attn, in_maps, trace=_trace)
    if _times is not None:
        _times.append(res2.exec_time_ns)

    out = np.empty((B, S, E), dtype=np.float32)
    for c in range(NCORES):
        out[BPC * c : BPC * (c + 1)] = res2.results[c]["out"].reshape(BPC, S, E)
    return out

